# revision 25
# baseline (speedup 1.0000x reference)
"""Trainium2 Bass kernel: Convpass adapter with hypernet-generated 3x3 conv.

Per core (data-parallel over batch, 8 images/core):
  hypernet: conv_w = emb @ w_hyper + b_hyper, via the diag-window matmul
            trick with both o-halves packed on 128 partitions (64 matmuls
            of N=288). The 4.7MB bf16 w_hyper streams in 8 DMA chunks with
            matmuls chasing the chunks; w_conv2 is built in two 32-row
            groups overlapped with the stream.
  down:     xT[128c,4k,784] @ [w_down|w_down] -> psum [128, 392] per half
            (x arrives pre-transposed bf16 from the host)
  gelu1:    quickgelu(x+b) as ONE scalar activation per half from psum
            rows 0-63 into the padded A-top region; the three other
            conv-packing copies are derived from A-top:
              A-bottom  = A-top shifted by 1 flat element  (packs dx-pairs)
              A2-top    = A-top verbatim                   (gpsimd copy)
              A2-bottom = A-top shifted by 60 flat elements (packs dy-pairs)
            The two shifts are single CONTIGUOUS whole-image SBUF->SBUF
            DMAs (64 fat descriptors each) because a row/col shift in the
            padded [30,30] layout is a pure flat-offset shift; the pad
            columns wrap into each other and carry zeros. The previous
            per-(row,half) sliced DMA form generated 832 56-byte
            descriptors per call (13K packets/kernel) which saturated the
            DMA engines and starved the PE into HAM-throttled 1.2GHz.
  conv:     3x3 as 3 K=128 matmuls (dy-pairs on A2/w_conv6) + 1 K=128
            (dx-pair on A/w_conv2) + 1 K=64 (lone tap)
  gelu2:    quickgelu(scale*y) as one activation per half into y_act
  up:       out^T[128c,392] = w_up65[:,cslice].T @ y_act  (stationary w_up,
            ones-row fused bias); stored transposed bf16, host untransposes.

Engine budget per image (targeting warm-PE 4.2us/image):
  PE 26 matmuls; scalar 4 acts; vector 8 psum->sbuf casts; gpsimd pad
  memsets + A2-top copy + y_act ones row; sync queue all DMA triggers
  except A2-bottom (scalar).

All small constants are assembled host-side in bf16 and loaded as one
contiguous tensor over the fast hardware DGE ring. The image loop is
software-pipelined for the in-order PE queue: tensor order is conv(i),
down(i+2), up(i), with gelu1(i+2) issued right after down(i+2).
"""

import os

import numpy as np
import ml_dtypes

import concourse.bass as bass
import concourse.mybir as mybir
import concourse.tile as tile
from concourse import bacc
from concourse.bass_utils import run_bass_kernel_spmd

# Problem shapes (hardcoded per contract).
B, H, W, C = 64, 28, 28, 512
DIM, EMB = 64, 64
NCORES = 8
B_LOC = B // NCORES            # 8 images per core
PIX = H * W                    # 784 pixels per image
PW = W + 2                     # 30 padded width
PAD = PW * (H + 2)             # 900 padded pixels per image
RH = 2                         # row-halves per image
RROWS = H // RH                # 14 rows per half
NHALF = RROWS * W              # 392 pixels per half-tile
KCH = C // 128                 # 4 contraction chunks of 128 channels
JTOT = DIM * DIM * 9           # 36864 hypernet outputs
NHYP = 32 * 9                  # 288 = free size of packed hypernet matmuls

NCHUNK = 8                     # w_hyper streaming DMA chunks
ICH = DIM // NCHUNK            # 8 i-rows per DMA chunk
IGRP = 32                      # i-rows per compute/build group (32-aligned)

# packed-const column offsets (bf16 [128, CPACK_W])
CP_WDOWN = 0                   # [128, 512]  w_down duplicated, (k m) layout
CP_WUP = 512                   # [65, 512]   w_up with bias row 64
CP_T2 = 1024                   # [128, 192]  hypernet lhsT window tensor
CP_BPREP = 1216                # [128, 288]  conv bias, psum-row layout
CP_SEL = 1504                  # [128, 128] shift-up-64 + [64, 128] shift-dn-64
CPACK_W = 1760

F32 = mybir.dt.float32
BF16 = mybir.dt.bfloat16
GELU_A = 1.702
# CoreSim doesn't implement Gelu_apprx_sigmoid; substitute Sigmoid for
# structure-only sim runs (numerics then checked on HW via --randup).
ACT_QGELU = (
    mybir.ActivationFunctionType.Sigmoid
    if os.environ.get("KERNEL_DEBUG_SIM_ACT") == "1"
    else mybir.ActivationFunctionType.Gelu_apprx_sigmoid
)

_CACHE = {}


def build_kernel():
    if "nc" in _CACHE:
        return _CACHE["nc"]

    nc = bacc.Bacc("TRN2", target_bir_lowering=False, debug=False)

    x_d = nc.dram_tensor("x", [B_LOC, 128, KCH * PIX], BF16, kind="ExternalInput")
    cpk_d = nc.dram_tensor("cpack", [128, CPACK_W], BF16, kind="ExternalInput")
    cf_d = nc.dram_tensor("cf32", [128, 2], F32, kind="ExternalInput")
    # host-packed hypernet: [128, i, ol, t]; rows 0-63 = o<32, 64-127 = o>=32
    wh_d = nc.dram_tensor("w_hyper", [128, DIM * NHYP], BF16, kind="ExternalInput")
    out_d = nc.dram_tensor("out", [B_LOC, 128, KCH * PIX], BF16, kind="ExternalOutput")

    with tile.TileContext(nc) as tc:
        with tc.tile_pool(name="consts", bufs=1) as consts:
            # ---- constants: ONE cpack DMA (0.38MB, first on sync) + cf32;
            # everything else is a view into cpk_sb ----
            cpk_sb = consts.tile([128, CPACK_W], BF16)
            nc.sync.dma_start(cpk_sb[:], cpk_d[:])
            cf_sb = consts.tile([128, 2], F32)
            nc.sync.dma_start(cf_sb[:], cf_d[:])
            t2 = cpk_sb[:, CP_T2 : CP_T2 + 192]
            w_down2 = cpk_sb[:, CP_WDOWN : CP_WDOWN + 512].rearrange(
                "p (k m) -> p k m", k=KCH
            )
            b_prep2 = cpk_sb[:, CP_BPREP : CP_BPREP + NHYP]
            w_up65 = cpk_sb[0 : DIM + 1, CP_WUP : CP_WUP + 512]
            b_down2 = cf_sb[:, 0:1]
            scale_sb = cf_sb[0:DIM, 1:2]
            # 0/1 selector stationaries: matmuls route partitions, so the
            # hypernet-build partition moves never touch a DMA ring (ring
            # transfers were landing ~9us late behind the x-image backlog)
            sel_up = cpk_sb[:, CP_SEL : CP_SEL + 128]
            sel_dn = cpk_sb[0:DIM, CP_SEL + 128 : CP_SEL + 256]

            # PE warm-up: the HAM clock gate holds the PE at 1.2GHz until it
            # sees ~3.4us of sustained busy. Burn that window on dummy
            # matmuls over a scratch tile while the prologue DMAs stream, so
            # the hypernet and image matmuls all run at 2.4GHz. The two tiny
            # dummy activations preload the Gelu/Copy LUTs so the 1.3us
            # ACT_TABLE_LOADs don't land in the act critical path later.
            scratch = consts.tile([128, 128], BF16)
            nc.vector.memset(scratch[:], 0.25)
            dum = consts.tile([1, 16], BF16)
            nc.scalar.activation(
                dum[:], scratch[0:1, 0:16], ACT_QGELU, bias=0.0, scale=1.0
            )
            nc.scalar.copy(dum[:], scratch[0:1, 0:16])

            # w_conv2 top rows: W[i, (h, t, o32)] assembled from the
            # hypernet psum; only rows 0-63 are ever written/read (the
            # tap-pair stationaries live in wtap below)
            w_conv2 = consts.tile([128, DIM * 9], BF16)

            # ---- main pools ----
            with (
                tc.tile_pool(name="whpool", bufs=1) as whpool,
                tc.tile_pool(name="xin", bufs=4) as xin,
                tc.tile_pool(name="xact", bufs=4) as xactp,
                tc.tile_pool(name="xact2", bufs=4) as xact2p,
                tc.tile_pool(name="yact", bufs=3) as yactp,
                tc.tile_pool(name="tmp", bufs=6) as tmpp,
                tc.tile_pool(name="outs", bufs=2) as outsp,
                tc.tile_pool(name="ps_s", bufs=2, space="PSUM") as ps_sp,
                tc.tile_pool(name="ps_c", bufs=2, space="PSUM") as ps_cp,
                tc.tile_pool(name="ps_u", bufs=3, space="PSUM") as ps_up,
                tc.tile_pool(name="ps_h", bufs=1, space="PSUM") as ps_hp,
            ):
                # ---- prologue ----
                # warm-up burn (~3.6us of PE busy) in a dedicated psum bank
                # so the WAW chain of later heartbeats never blocks the
                # rotating up-proj psum tiles
                ps_w = ps_hp.tile([128, 128], F32, tag="hb", name="warm")
                for _ in range(34):
                    nc.tensor.matmul(
                        ps_w[:], scratch[:], scratch[:],
                        start=True, stop=True,
                    )

                def heartbeat(rhs64):
                    """Tiny matmul whose rhs is freshly-DMAed data: fires
                    exactly when that transfer lands, keeping the HAM
                    activity window non-idle through DMA-bound stretches
                    (an idle >3.4us re-throttles the PE to 1.2GHz, and it
                    has been observed stuck cold for 25us+ afterwards)."""
                    nc.tensor.matmul(
                        ps_w[0:64, 0:64], scratch[:, 0:64], rhs64,
                        start=True, stop=True,
                    )

                # one tile PER w_hyper chunk: tile-granular dependency
                # tracking would stall the first hypernet matmul until the
                # last chunk landed if this were a single tile
                def load_x(img, eng=None):
                    xT = xin.tile([128, KCH, PIX], BF16, tag="x", name=f"x{img}")
                    (eng or nc.sync).dma_start(
                        xT[:].rearrange("p k n -> p (k n)"), x_d[img]
                    )
                    heartbeat(xT[:, 0, 0:64])
                    return xT

                # queue split (each HWDGE data ring drains in FIFO
                # order, and triggers enter a ring the moment their deps
                # are ready -- so big loads must not be able to jump in
                # front of latency-critical small transfers):
                #   scalar ring: ALL wh chunks (drained ~27us), then only
                #     the small shift/build transfers (land promptly)
                #   sync ring:   cpack, cf32, x0.., stores (big streams)
                xTs = [load_x(0)]
                wh_chunks = []
                for q in range(NCHUNK):
                    cw = ICH * NHYP
                    t = whpool.tile([128, cw], BF16, tag=f"wh{q}")
                    eng = nc.scalar if q % 2 == 0 else nc.sync
                    eng.dma_start(t[:], wh_d[:, q * cw : (q + 1) * cw])
                    heartbeat(t[:, 0:64])
                    wh_chunks.append(t)
                xTs.append(load_x(1))
                xTs.append(load_x(2))

                def make_xacts(img):
                    """A/A2 padded buffers; one full memset of A-top zeroes
                    the pads (strided pad-only memsets measured slower on
                    gpsimd than one contiguous sweep); the act overwrites
                    the interior, the derived copies carry pads along."""
                    x_act = xactp.tile([128, PAD], BF16, tag="xa", name=f"xa{img}")
                    nc.gpsimd.memset(x_act[0:DIM, :], 0.0)
                    x_act2 = xact2p.tile([128, PAD], BF16, tag="xb", name=f"xb{img}")
                    return (x_act, x_act2)

                xacts = [make_xacts(0), make_xacts(1)]

                def wh_slice(il):
                    """rhs [128, 288] for hypernet row il, from its chunk."""
                    t = wh_chunks[il // ICH]
                    j = il % ICH
                    return t[:, j * NHYP : (j + 1) * NHYP]

                def down(img, xT):
                    """down-proj matmuls -> 2 psum tiles [128, 392]"""
                    ps_ds = [
                        ps_sp.tile([128, NHALF], F32, tag="pss", name=f"psd{img}_{rh}")
                        for rh in range(RH)
                    ]
                    for k in range(KCH):
                        for rh in range(RH):
                            nc.tensor.matmul(
                                ps_ds[rh][:],
                                w_down2[:, k, :],
                                xT[:, k, rh * NHALF : (rh + 1) * NHALF],
                                start=(k == 0),
                                stop=(k == KCH - 1),
                            )
                    return ps_ds

                def gelu1(img, ps_ds, x_act, x_act2):
                    """quickgelu(x+b) = Gelu_apprx_sigmoid(1.0*x + b) from
                    psum rows 0-63 into A-top (padded rows 1..28, cols
                    1..28)."""
                    x_act_v = x_act[:].rearrange("d (r c) -> d r c", c=PW)
                    for rh in range(RH):
                        ps_v = ps_ds[rh][:].rearrange("d (r c) -> d r c", c=W)
                        rows = slice(1 + rh * RROWS, 1 + (rh + 1) * RROWS)
                        nc.scalar.activation(
                            x_act_v[:DIM, rows, 1 : 1 + W],
                            ps_v[:DIM],
                            ACT_QGELU,
                            bias=b_down2[:DIM],
                            scale=1.0,
                        )
                    return (x_act_v, x_act2[:].rearrange("d (r c) -> d r c", c=PW))

                def shifts(img, x_act, x_act2):
                    """Derive the three other conv-packing copies from
                    A-top. conv(img) is ~1.5 image-blocks away, so these
                    can trail the gelu2/up work in each queue."""
                    # A-bottom: act shifted one flat element (one padded
                    # col); pad cols wrap into each other and carry zeros
                    nc.scalar.dma_start(
                        x_act[DIM:, 0 : PAD - 1], x_act[:DIM, 1:PAD]
                    )
                    # A2-top duplicates A-top verbatim (pads included);
                    # vector: gpsimd measured 3.2us for this copy vs 0.9
                    nc.vector.tensor_copy(x_act2[:DIM, :], x_act[:DIM, :])
                    # A2-bottom: act shifted one padded row-pair (2 rows =
                    # 60 flat elements)
                    nc.scalar.dma_start(
                        x_act2[DIM:, 0 : PAD - 2 * PW], x_act[:DIM, 2 * PW : PAD]
                    )

                # Prologue: downs go AFTER the first hypernet group in the
                # in-order PE queue (a stalled down matmul ahead of the
                # hypernet delays conv0 behind it).
                downed = []

                def issue_down(img, with_shifts=True):
                    ps = down(img, xTs[img])
                    xa = gelu1(img, ps, *xacts[img])
                    if with_shifts:
                        shifts(img, *xacts[img])
                    downed.append((ps, xa))

                # full hypernet: matmuls chase the streaming w_hyper DMA
                # chunk by chunk (region-level deps); psum/build work in two
                # 32-row groups (engine partition slices need 32 alignment).
                # Group g's psum rows [32g,32g+32) = W[i, o<32], rows
                # [64+32g, ..) = W[i, o>=32].
                t_b = tmpp.tile([128, NHYP], BF16, tag="t", name="t_b")
                for g in range(DIM // IGRP):
                    ps_q = ps_up.tile([128, NHYP], F32, tag="psu", name=f"hyp{g}")
                    for il in range(g * IGRP, (g + 1) * IGRP):
                        nc.tensor.matmul(
                            ps_q[:],
                            t2[:, 64 - il : 192 - il],
                            wh_slice(il),
                            start=(il % IGRP == 0),
                            stop=(il % IGRP == IGRP - 1),
                        )
                    if g == 0:
                        # two images' downs fill the w_hyper chunk-wait gaps
                        # between the hypernet groups
                        issue_down(0)
                        issue_down(1)
                    rt = slice(g * IGRP, (g + 1) * IGRP)
                    rb = slice(DIM + g * IGRP, DIM + (g + 1) * IGRP)
                    nc.vector.tensor_tensor(
                        w_conv2[rt, :NHYP], ps_q[rt, :], b_prep2[rt, :],
                        mybir.AluOpType.add,
                    )
                    nc.vector.tensor_tensor(
                        t_b[rb, :], ps_q[rb, :], b_prep2[rb, :],
                        mybir.AluOpType.add,
                    )
                # o>=32 block: PE routes t_b partitions 64+r -> r (matmul
                # operands must start at partition 0/32/64, so one K=64
                # routing matmul covers both groups), vector drains psum
                # into w_conv2 cols 288:576
                ps_m = ps_up.tile([128, NHYP], F32, tag="psu", name="mv")
                nc.tensor.matmul(
                    ps_m[:], sel_up[DIM:, :], t_b[DIM:, :],
                    start=True, stop=True,
                )
                nc.vector.tensor_copy(w_conv2[0:DIM, NHYP:], ps_m[0:DIM, :])
                # PE routes w_conv2 top rows r -> 64+r into psum (2 col
                # halves), then the 5 tap-pair stationaries are assembled
                # straight from SBUF-top/psum-bottom: wtap block rows 0-63 =
                # tap tA, rows 64-127 = tap tB (the pair contracted by one
                # K=128 matmul). 0..2 = (0,dx)+(2,dx), 3 = (1,0)+(1,1),
                # 4 = lone (1,2) (top half only). matmul lhsT APs may only
                # have ONE free dim, so these are contiguous [128, 64].
                ps_d = []
                for hh in range(2):
                    p = ps_up.tile([128, NHYP], F32, tag="psu", name=f"dn{hh}")
                    nc.tensor.matmul(
                        p[:], sel_dn[:, :],
                        w_conv2[0:DIM, hh * NHYP : (hh + 1) * NHYP],
                        start=True, stop=True,
                    )
                    ps_d.append(p)
                w2t = w_conv2[0:DIM, :].rearrange("p (h t o) -> p h t o", h=2, t=9)
                wtap = consts.tile([128, 5, 64], BF16)
                for j, (tA, tB) in enumerate(
                    [(0, 6), (1, 7), (2, 8), (3, 4), (5, None)]
                ):
                    nc.vector.tensor_copy(
                        wtap[0:DIM, j, :].rearrange("p (h o) -> p h o", h=2),
                        w2t[:, :, tA, :],
                    )
                    if tB is None:
                        continue
                    wb_v = wtap[DIM:, j, :].rearrange("p (h o) -> p h o", h=2)
                    for hh in range(2):
                        nc.vector.tensor_copy(
                            wb_v[:, hh, :],
                            ps_d[hh][DIM:, tB * 32 : (tB + 1) * 32],
                        )

                def conv(img):
                    """conv, 5 matmuls per half: 3 vertical pairs
                    (0,dx)+(2,dx) on A2/w_conv6, the pair (1,0)+(1,1) on
                    A/w_conv2, and the lone (1,2) tap at K=64"""
                    xact_cur, xact2_cur = downed[img][1]
                    ps_cs = []
                    for rh in range(RH):
                        ps_c = ps_cp.tile(
                            [DIM, NHALF], F32, tag="psc", name=f"psc{img}_{rh}"
                        )
                        for dx in range(3):
                            src = xact2_cur[
                                :, rh * RROWS : rh * RROWS + RROWS, dx : dx + W
                            ]
                            nc.tensor.matmul(
                                ps_c[:],
                                wtap[:, dx, :],
                                src,
                                start=(dx == 0),
                                stop=False,
                            )
                        nc.tensor.matmul(
                            ps_c[:],
                            wtap[:, 3, :],
                            xact_cur[
                                :, rh * RROWS + 1 : rh * RROWS + 1 + RROWS, 0:W
                            ],
                            start=False,
                            stop=False,
                        )
                        nc.tensor.matmul(
                            ps_c[:],
                            wtap[:DIM, 4, :],
                            xact_cur[
                                :DIM,
                                rh * RROWS + 1 : rh * RROWS + 1 + RROWS,
                                2 : 2 + W,
                            ],
                            start=False,
                            stop=True,
                        )
                        ps_cs.append(ps_c)
                    return ps_cs

                def up_part(img, o_sb, y_act, half, alternate=False):
                    """4 up matmuls (2 c-chunks) + psum->sbuf casts."""
                    for j, (kc, rh) in enumerate(
                        (kc, rh)
                        for kc in ((0, 1) if half == 0 else (2, 3))
                        for rh in range(RH)
                    ):
                        ps_u = ps_up.tile(
                            [128, NHALF], F32, tag="psu", name=f"psu{img}_{kc}_{rh}"
                        )
                        nc.tensor.matmul(
                            ps_u[:],
                            w_up65[:, kc * 128 : (kc + 1) * 128],
                            y_act[:, rh * NHALF : (rh + 1) * NHALF],
                            start=True,
                            stop=True,
                        )
                        dst = o_sb[:, kc, rh * NHALF : (rh + 1) * NHALF]
                        if (alternate and j % 2 == 1) or (
                            not alternate and half == 1 and kc == 3 and rh == 1
                        ):
                            nc.scalar.copy(dst, ps_u[:])
                        else:
                            nc.vector.tensor_copy(dst, ps_u[:])

                # Steady-state block for image i (software-pipelined):
                #   PE:     up_a(i-1), conv(i), up_b(i-1), down(i+2)
                #   scalar: gelu2(i), gelu1(i+2), A2-bottom trigger(i+2)
                #   vector: casts(i-1)
                #   gpsimd: y-ones(i), pad memsets(i+2), A2-top copy(i+2)
                #   sync:   load(i+3), store(i-1), A-bottom trigger(i+2)
                # Splitting up(i-1) around conv(i) gives the vector casts a
                # conv's worth of slack before the second psum rotation, so
                # the in-order PE queue never waits on a cast.
                y_acts = {}
                o_sbs = {}
                for img in range(B_LOC):
                    if img + 3 < B_LOC:
                        xTs.append(load_x(img + 3))
                    if img >= 1:
                        o_sbs[img - 1] = outsp.tile(
                            [128, KCH, PIX], BF16, tag="o", name=f"o{img-1}"
                        )
                        up_part(img - 1, o_sbs[img - 1], y_acts[img - 1], 0)

                    ps_cs = conv(img)

                    # gelu2: quickgelu(scale*y) = Gelu_apprx_sigmoid(scale*y)
                    # straight from psum into y_act (ones row fuses up bias)
                    y_act = yactp.tile([DIM + 1, PIX], BF16, tag="ya")
                    y_acts[img] = y_act
                    nc.gpsimd.memset(y_act[DIM : DIM + 1, :], 1.0)
                    for rh in range(RH):
                        nc.scalar.activation(
                            y_act[:DIM, rh * NHALF : (rh + 1) * NHALF],
                            ps_cs[rh][:],
                            ACT_QGELU,
                            bias=0.0,
                            scale=scale_sb[:],
                        )

                    if img >= 1:
                        up_part(img - 1, o_sbs[img - 1], y_acts[img - 1], 1)
                        nc.sync.dma_start(
                            out_d[img - 1][:],
                            o_sbs[img - 1][:].rearrange("p k n -> p (k n)"),
                        )

                    if img + 2 < B_LOC:
                        xacts.append(make_xacts(img + 2))
                        issue_down(img + 2)

                # drain: last image's up with casts split across vector and
                # scalar (both idle now), store in two halves so the first
                # starts before the second is cast
                li = B_LOC - 1
                o_sb = outsp.tile([128, KCH, PIX], BF16, tag="o")
                up_part(li, o_sb, y_acts[li], 0, alternate=True)
                nc.sync.dma_start(
                    out_d[li][:, : 2 * PIX],
                    o_sb[:, 0:2, :].rearrange("p k n -> p (k n)"),
                )
                up_part(li, o_sb, y_acts[li], 1, alternate=True)
                nc.sync.dma_start(
                    out_d[li][:, 2 * PIX :],
                    o_sb[:, 2:4, :].rearrange("p k n -> p (k n)"),
                )

    nc.compile()
    _CACHE["nc"] = nc
    return nc


def _make_in_maps(inputs):
    bf16 = ml_dtypes.bfloat16
    x = np.ascontiguousarray(inputs["x"], dtype=np.float32)

    # ---- packed bf16 consts ----
    cpk = np.zeros((128, CPACK_W), dtype=bf16)
    wd = np.asarray(inputs["w_down"], np.float32).astype(bf16)
    t = wd.reshape(KCH, 128, DIM).transpose(1, 0, 2)       # [p, k, d]
    cpk[:, CP_WDOWN : CP_WDOWN + 512] = np.concatenate(
        [t, t], axis=2
    ).reshape(128, 512)
    cpk[0:DIM, CP_WUP : CP_WUP + 512] = np.asarray(
        inputs["w_up"], np.float32
    ).astype(bf16)
    cpk[DIM, CP_WUP : CP_WUP + 512] = np.asarray(
        inputs["b_up"], np.float32
    ).astype(bf16)
    emb = np.asarray(inputs["layer_emb"], np.float32).astype(bf16)
    cpk[0:EMB, CP_T2 + 64] = emb
    cpk[EMB:128, CP_T2 + 128] = emb
    bh = np.asarray(inputs["b_hyper"], np.float32).reshape(DIM, DIM, 9)
    b_ot = bh.transpose(1, 0, 2).astype(bf16)              # [i, o, t]
    cpk[0:DIM, CP_BPREP : CP_BPREP + NHYP] = (
        b_ot[:, :32].transpose(0, 2, 1).reshape(DIM, NHYP)   # [i, (t, ol)]
    )
    cpk[DIM:, CP_BPREP : CP_BPREP + NHYP] = (
        b_ot[:, 32:].transpose(0, 2, 1).reshape(DIM, NHYP)
    )

    sel = np.zeros((128, 256), dtype=bf16)
    for m in range(DIM):
        sel[m + DIM, m] = 1.0           # shift-up-64:  out row m <- row m+64
    for p in range(DIM):
        sel[p, 128 + DIM + p] = 1.0     # shift-dn-64:  out row p+64 <- row p
    cpk[:, CP_SEL:] = sel

    cf = np.zeros((128, 2), np.float32)
    bd = np.asarray(inputs["b_down"], np.float32)
    cf[0:DIM, 0] = bd
    cf[DIM:, 0] = bd
    cf[0:DIM, 1] = np.asarray(inputs["scale"], np.float32)

    # ---- packed hypernet: [128, i, ol, t]; rows 0-63 = o<32 block ----
    wh = np.asarray(inputs["w_hyper"], np.float32).astype(bf16)
    wh = wh.reshape(EMB, DIM, DIM, 9)                      # [e, o, i, t]
    top = wh[:, :32].transpose(0, 2, 3, 1)                 # [e, i, t, ol]
    bot = wh[:, 32:].transpose(0, 2, 3, 1)
    whp = np.ascontiguousarray(
        np.concatenate([top, bot], axis=0).reshape(128, DIM * NHYP)
    )

    shared = {"cpack": cpk, "cf32": cf, "w_hyper": whp}
    in_maps = []
    for c in range(NCORES):
        xc = x[c * B_LOC : (c + 1) * B_LOC].reshape(B_LOC, PIX, KCH, 128)
        xt = np.ascontiguousarray(xc.transpose(0, 3, 2, 1)).astype(bf16)
        in_maps.append({"x": xt.reshape(B_LOC, 128, KCH * PIX), **shared})
    return in_maps


def _untranspose_out(res):
    outs = []
    for c in range(NCORES):
        o = np.asarray(res.results[c]["out"]).reshape(B_LOC, 128, KCH, PIX)
        o = o.transpose(0, 3, 2, 1).astype(np.float32)  # [img, pix, kc, p]
        outs.append(o.reshape(B_LOC, H, W, C))
    return np.concatenate(outs, axis=0)


def kernel(**inputs) -> np.ndarray:
    nc = build_kernel()
    in_maps = _make_in_maps(inputs)
    res = run_bass_kernel_spmd(nc, in_maps, core_ids=list(range(NCORES)))
    return _untranspose_out(res)


def run_traced(inputs, **kw):
    """For test.py: run with tracing to get HW exec time."""
    nc = build_kernel()
    in_maps = _make_in_maps(inputs)
    return run_bass_kernel_spmd(
        nc, in_maps, core_ids=list(range(NCORES)), trace=True, **kw
    )


# revision 26
# speedup vs baseline: 1.0349x; 1.0349x over previous
"""Trainium2 Bass kernel: Convpass adapter with hypernet-generated 3x3 conv.

Per core (data-parallel over batch, 8 images/core):
  hypernet: conv_w = emb @ w_hyper + b_hyper, via the diag-window matmul
            trick with both o-halves packed on 128 partitions (64 matmuls
            of N=288). The 4.7MB bf16 w_hyper streams in 8 DMA chunks with
            matmuls chasing the chunks; w_conv2 is built in two 32-row
            groups overlapped with the stream.
  down:     xT[128c,4k,784] @ [w_down|w_down] -> psum [128, 392] per half
            (x arrives pre-transposed bf16 from the host)
  gelu1:    quickgelu(x+b) as ONE scalar activation per half from psum
            rows 0-63 into the padded A-top region; the three other
            conv-packing copies are derived from A-top:
              A-bottom  = A-top shifted by 1 flat element  (packs dx-pairs)
              A2-top    = A-top verbatim                   (gpsimd copy)
              A2-bottom = A-top shifted by 60 flat elements (packs dy-pairs)
            The two shifts are single CONTIGUOUS whole-image SBUF->SBUF
            DMAs (64 fat descriptors each) because a row/col shift in the
            padded [30,30] layout is a pure flat-offset shift; the pad
            columns wrap into each other and carry zeros. The previous
            per-(row,half) sliced DMA form generated 832 56-byte
            descriptors per call (13K packets/kernel) which saturated the
            DMA engines and starved the PE into HAM-throttled 1.2GHz.
  conv:     3x3 as 3 K=128 matmuls (dy-pairs on A2/w_conv6) + 1 K=128
            (dx-pair on A/w_conv2) + 1 K=64 (lone tap)
  gelu2:    quickgelu(scale*y) as one activation per half into y_act
  up:       out^T[128c,392] = w_up65[:,cslice].T @ y_act  (stationary w_up,
            ones-row fused bias); stored transposed bf16, host untransposes.

Engine budget per image (targeting warm-PE 4.2us/image):
  PE 26 matmuls; scalar 4 acts; vector 8 psum->sbuf casts; gpsimd pad
  memsets + A2-top copy + y_act ones row; sync queue all DMA triggers
  except A2-bottom (scalar).

All small constants are assembled host-side in bf16 and loaded as one
contiguous tensor over the fast hardware DGE ring. The image loop is
software-pipelined for the in-order PE queue: tensor order is conv(i),
down(i+2), up(i), with gelu1(i+2) issued right after down(i+2).
"""

import os

import numpy as np
import ml_dtypes

import concourse.bass as bass
import concourse.mybir as mybir
import concourse.tile as tile
from concourse import bacc
from concourse.bass_utils import run_bass_kernel_spmd

# Problem shapes (hardcoded per contract).
B, H, W, C = 64, 28, 28, 512
DIM, EMB = 64, 64
NCORES = 8
B_LOC = B // NCORES            # 8 images per core
PIX = H * W                    # 784 pixels per image
PW = W + 2                     # 30 padded width
PAD = PW * (H + 2)             # 900 padded pixels per image
RH = 2                         # row-halves per image
RROWS = H // RH                # 14 rows per half
NHALF = RROWS * W              # 392 pixels per half-tile
KCH = C // 128                 # 4 contraction chunks of 128 channels
JTOT = DIM * DIM * 9           # 36864 hypernet outputs
NHYP = 32 * 9                  # 288 = free size of packed hypernet matmuls

NCHUNK = 8                     # w_hyper streaming DMA chunks
ICH = DIM // NCHUNK            # 8 i-rows per DMA chunk
IGRP = 32                      # i-rows per compute/build group (32-aligned)

# packed-const column offsets (bf16 [128, CPACK_W])
CP_WDOWN = 0                   # [128, 512]  w_down duplicated, (k m) layout
CP_WUP = 512                   # [65, 512]   w_up with bias row 64
CP_T2 = 1024                   # [128, 192]  hypernet lhsT window tensor
CP_BPREP = 1216                # [128, 288]  conv bias, psum-row layout
CP_SEL = 1504                  # [128, 128] shift-up-64 + [64, 128] shift-dn-64
CPACK_W = 1760

F32 = mybir.dt.float32
BF16 = mybir.dt.bfloat16
GELU_A = 1.702
# CoreSim doesn't implement Gelu_apprx_sigmoid; substitute Sigmoid for
# structure-only sim runs (numerics then checked on HW via --randup).
ACT_QGELU = (
    mybir.ActivationFunctionType.Sigmoid
    if os.environ.get("KERNEL_DEBUG_SIM_ACT") == "1"
    else mybir.ActivationFunctionType.Gelu_apprx_sigmoid
)

_CACHE = {}


def build_kernel():
    if "nc" in _CACHE:
        return _CACHE["nc"]

    nc = bacc.Bacc("TRN2", target_bir_lowering=False, debug=False)

    x_d = nc.dram_tensor("x", [B_LOC, 128, KCH * PIX], BF16, kind="ExternalInput")
    cpk_d = nc.dram_tensor("cpack", [128, CPACK_W], BF16, kind="ExternalInput")
    cf_d = nc.dram_tensor("cf32", [128, 6], F32, kind="ExternalInput")
    # host-packed hypernet: [128, i, ol, t]; rows 0-63 = o<32, 64-127 = o>=32
    wh_d = nc.dram_tensor("w_hyper", [128, DIM * NHYP], BF16, kind="ExternalInput")
    out_d = nc.dram_tensor("out", [B_LOC, 128, KCH * PIX], BF16, kind="ExternalOutput")

    with tile.TileContext(nc) as tc:
        with tc.tile_pool(name="consts", bufs=1) as consts:
            # ---- constants: ONE cpack DMA (0.38MB, first on sync) + cf32;
            # everything else is a view into cpk_sb ----
            cpk_sb = consts.tile([128, CPACK_W], BF16)
            nc.sync.dma_start(cpk_sb[:], cpk_d[:])
            cf_sb = consts.tile([128, 6], F32)
            nc.sync.dma_start(cf_sb[:], cf_d[:])
            t2 = cpk_sb[:, CP_T2 : CP_T2 + 192]
            w_down2 = cpk_sb[:, CP_WDOWN : CP_WDOWN + 512].rearrange(
                "p (k m) -> p k m", k=KCH
            )
            b_prep2 = cpk_sb[:, CP_BPREP : CP_BPREP + NHYP]
            w_up65 = cpk_sb[0 : DIM + 1, CP_WUP : CP_WUP + 512]
            b_down2 = cf_sb[:, 0:1]
            scale_sb = cf_sb[0:DIM, 1:2]
            b_up_c = [cf_sb[:, 2 + kc : 3 + kc] for kc in range(KCH)]
            # 0/1 selector stationaries: matmuls route partitions, so the
            # hypernet-build partition moves never touch a DMA ring (ring
            # transfers were landing ~9us late behind the x-image backlog)
            sel_up = cpk_sb[:, CP_SEL : CP_SEL + 128]
            sel_dn = cpk_sb[0:DIM, CP_SEL + 128 : CP_SEL + 256]

            # PE warm-up: the HAM clock gate holds the PE at 1.2GHz until it
            # sees ~3.4us of sustained busy. Burn that window on dummy
            # matmuls over a scratch tile while the prologue DMAs stream, so
            # the hypernet and image matmuls all run at 2.4GHz. The two tiny
            # dummy activations preload the Gelu/Copy LUTs so the 1.3us
            # ACT_TABLE_LOADs don't land in the act critical path later.
            scratch = consts.tile([128, 128], BF16)
            nc.vector.memset(scratch[:], 0.25)
            dum = consts.tile([1, 16], BF16)
            nc.scalar.activation(
                dum[:], scratch[0:1, 0:16], ACT_QGELU, bias=0.0, scale=1.0
            )
            nc.scalar.copy(dum[:], scratch[0:1, 0:16])

            # w_conv2 top rows: W[i, (h, t, o32)] assembled from the
            # hypernet psum; only rows 0-63 are ever written/read (the
            # tap-pair stationaries live in wtap below)
            w_conv2 = consts.tile([128, DIM * 9], BF16)

            # ---- main pools ----
            with (
                tc.tile_pool(name="whpool", bufs=1) as whpool,
                tc.tile_pool(name="xin", bufs=4) as xin,
                tc.tile_pool(name="xact", bufs=4) as xactp,
                tc.tile_pool(name="xact2", bufs=4) as xact2p,
                tc.tile_pool(name="yact", bufs=3) as yactp,
                tc.tile_pool(name="tmp", bufs=6) as tmpp,
                tc.tile_pool(name="outs", bufs=2) as outsp,
                tc.tile_pool(name="ps_s", bufs=2, space="PSUM") as ps_sp,
                tc.tile_pool(name="ps_c", bufs=2, space="PSUM") as ps_cp,
                tc.tile_pool(name="ps_u", bufs=3, space="PSUM") as ps_up,
                tc.tile_pool(name="ps_h", bufs=1, space="PSUM") as ps_hp,
            ):
                # ---- prologue ----
                # warm-up burn (~3.6us of PE busy) in a dedicated psum bank
                # so the WAW chain of later heartbeats never blocks the
                # rotating up-proj psum tiles
                ps_w = ps_hp.tile([128, 128], F32, tag="hb", name="warm")
                for _ in range(34):
                    nc.tensor.matmul(
                        ps_w[:], scratch[:], scratch[:],
                        start=True, stop=True,
                    )

                def heartbeat(rhs64):
                    """Tiny matmul whose rhs is freshly-DMAed data: fires
                    exactly when that transfer lands, keeping the HAM
                    activity window non-idle through DMA-bound stretches
                    (an idle >3.4us re-throttles the PE to 1.2GHz, and it
                    has been observed stuck cold for 25us+ afterwards)."""
                    nc.tensor.matmul(
                        ps_w[0:64, 0:64], scratch[:, 0:64], rhs64,
                        start=True, stop=True,
                    )

                # one tile PER w_hyper chunk: tile-granular dependency
                # tracking would stall the first hypernet matmul until the
                # last chunk landed if this were a single tile
                def load_x(img, eng=None):
                    xT = xin.tile([128, KCH, PIX], BF16, tag="x", name=f"x{img}")
                    (eng or nc.sync).dma_start(
                        xT[:].rearrange("p k n -> p (k n)"), x_d[img]
                    )
                    heartbeat(xT[:, 0, 0:64])
                    return xT

                # queue split (each HWDGE data ring drains in FIFO
                # order, and triggers enter a ring the moment their deps
                # are ready -- so big loads must not be able to jump in
                # front of latency-critical small transfers):
                #   scalar ring: ALL wh chunks (drained ~27us), then only
                #     the small shift/build transfers (land promptly)
                #   sync ring:   cpack, cf32, x0.., stores (big streams)
                xTs = [load_x(0)]
                wh_chunks = []
                for q in range(NCHUNK):
                    cw = ICH * NHYP
                    t = whpool.tile([128, cw], BF16, tag=f"wh{q}")
                    eng = nc.scalar if q % 2 == 0 else nc.sync
                    eng.dma_start(t[:], wh_d[:, q * cw : (q + 1) * cw])
                    heartbeat(t[:, 0:64])
                    wh_chunks.append(t)
                xTs.append(load_x(1))
                xTs.append(load_x(2))

                def make_xacts(img):
                    """A/A2 padded buffers; one full memset of A-top zeroes
                    the pads (strided pad-only memsets measured slower on
                    gpsimd than one contiguous sweep); the act overwrites
                    the interior, the derived copies carry pads along."""
                    x_act = xactp.tile([128, PAD], BF16, tag="xa", name=f"xa{img}")
                    nc.gpsimd.memset(x_act[0:DIM, :], 0.0)
                    x_act2 = xact2p.tile([128, PAD], BF16, tag="xb", name=f"xb{img}")
                    return (x_act, x_act2)

                xacts = [make_xacts(0), make_xacts(1)]

                def wh_slice(il):
                    """rhs [128, 288] for hypernet row il, from its chunk."""
                    t = wh_chunks[il // ICH]
                    j = il % ICH
                    return t[:, j * NHYP : (j + 1) * NHYP]

                def down(img, xT):
                    """down-proj matmuls -> 2 psum tiles [128, 392]"""
                    ps_ds = [
                        ps_sp.tile([128, NHALF], F32, tag="pss", name=f"psd{img}_{rh}")
                        for rh in range(RH)
                    ]
                    for k in range(KCH):
                        for rh in range(RH):
                            nc.tensor.matmul(
                                ps_ds[rh][:],
                                w_down2[:, k, :],
                                xT[:, k, rh * NHALF : (rh + 1) * NHALF],
                                start=(k == 0),
                                stop=(k == KCH - 1),
                            )
                    return ps_ds

                def gelu1(img, ps_ds, x_act, x_act2):
                    """quickgelu(x+b) = Gelu_apprx_sigmoid(1.0*x + b) from
                    psum rows 0-63 into A-top (padded rows 1..28, cols
                    1..28)."""
                    x_act_v = x_act[:].rearrange("d (r c) -> d r c", c=PW)
                    for rh in range(RH):
                        ps_v = ps_ds[rh][:].rearrange("d (r c) -> d r c", c=W)
                        rows = slice(1 + rh * RROWS, 1 + (rh + 1) * RROWS)
                        nc.scalar.activation(
                            x_act_v[:DIM, rows, 1 : 1 + W],
                            ps_v[:DIM],
                            ACT_QGELU,
                            bias=b_down2[:DIM],
                            scale=1.0,
                        )
                    return (x_act_v, x_act2[:].rearrange("d (r c) -> d r c", c=PW))

                def shifts(img, x_act, x_act2):
                    """Derive the three other conv-packing copies from
                    A-top. conv(img) is ~1.5 image-blocks away, so these
                    can trail the gelu2/up work in each queue."""
                    # A-bottom: act shifted one flat element (one padded
                    # col); pad cols wrap into each other and carry zeros
                    nc.scalar.dma_start(
                        x_act[DIM:, 0 : PAD - 1], x_act[:DIM, 1:PAD]
                    )
                    # A2-top duplicates A-top verbatim (pads included);
                    # vector: gpsimd measured 3.2us for this copy vs 0.9
                    nc.vector.tensor_copy(x_act2[:DIM, :], x_act[:DIM, :])
                    # A2-bottom: act shifted one padded row-pair (2 rows =
                    # 60 flat elements)
                    nc.scalar.dma_start(
                        x_act2[DIM:, 0 : PAD - 2 * PW], x_act[:DIM, 2 * PW : PAD]
                    )

                # Prologue: downs go AFTER the first hypernet group in the
                # in-order PE queue (a stalled down matmul ahead of the
                # hypernet delays conv0 behind it).
                downed = []

                def issue_down(img, with_shifts=True):
                    ps = down(img, xTs[img])
                    xa = gelu1(img, ps, *xacts[img])
                    if with_shifts:
                        shifts(img, *xacts[img])
                    downed.append((ps, xa))

                # full hypernet: matmuls chase the streaming w_hyper DMA
                # chunk by chunk (region-level deps); psum/build work in two
                # 32-row groups (engine partition slices need 32 alignment).
                # Group g's psum rows [32g,32g+32) = W[i, o<32], rows
                # [64+32g, ..) = W[i, o>=32].
                t_b = tmpp.tile([128, NHYP], BF16, tag="t", name="t_b")
                for g in range(DIM // IGRP):
                    ps_q = ps_up.tile([128, NHYP], F32, tag="psu", name=f"hyp{g}")
                    for il in range(g * IGRP, (g + 1) * IGRP):
                        nc.tensor.matmul(
                            ps_q[:],
                            t2[:, 64 - il : 192 - il],
                            wh_slice(il),
                            start=(il % IGRP == 0),
                            stop=(il % IGRP == IGRP - 1),
                        )
                    if g == 0:
                        # two images' downs fill the w_hyper chunk-wait gaps
                        # between the hypernet groups
                        issue_down(0)
                        issue_down(1)
                    rt = slice(g * IGRP, (g + 1) * IGRP)
                    rb = slice(DIM + g * IGRP, DIM + (g + 1) * IGRP)
                    nc.vector.tensor_tensor(
                        w_conv2[rt, :NHYP], ps_q[rt, :], b_prep2[rt, :],
                        mybir.AluOpType.add,
                    )
                    nc.vector.tensor_tensor(
                        t_b[rb, :], ps_q[rb, :], b_prep2[rb, :],
                        mybir.AluOpType.add,
                    )
                # o>=32 block: PE routes t_b partitions 64+r -> r (matmul
                # operands must start at partition 0/32/64, so one K=64
                # routing matmul covers both groups), vector drains psum
                # into w_conv2 cols 288:576
                ps_m = ps_up.tile([128, NHYP], F32, tag="psu", name="mv")
                nc.tensor.matmul(
                    ps_m[:], sel_up[DIM:, :], t_b[DIM:, :],
                    start=True, stop=True,
                )
                nc.vector.tensor_copy(w_conv2[0:DIM, NHYP:], ps_m[0:DIM, :])
                # PE routes w_conv2 top rows r -> 64+r into psum (2 col
                # halves), then the 5 tap-pair stationaries are assembled
                # straight from SBUF-top/psum-bottom: wtap block rows 0-63 =
                # tap tA, rows 64-127 = tap tB (the pair contracted by one
                # K=128 matmul). 0..2 = (0,dx)+(2,dx), 3 = (1,0)+(1,1),
                # 4 = lone (1,2) (top half only). matmul lhsT APs may only
                # have ONE free dim, so these are contiguous [128, 64].
                ps_d = []
                for hh in range(2):
                    p = ps_up.tile([128, NHYP], F32, tag="psu", name=f"dn{hh}")
                    nc.tensor.matmul(
                        p[:], sel_dn[:, :],
                        w_conv2[0:DIM, hh * NHYP : (hh + 1) * NHYP],
                        start=True, stop=True,
                    )
                    ps_d.append(p)
                w2t = w_conv2[0:DIM, :].rearrange("p (h t o) -> p h t o", h=2, t=9)
                wtap = consts.tile([128, 5, 64], BF16)
                for j, (tA, tB) in enumerate(
                    [(0, 6), (1, 7), (2, 8), (3, 4), (5, None)]
                ):
                    nc.vector.tensor_copy(
                        wtap[0:DIM, j, :].rearrange("p (h o) -> p h o", h=2),
                        w2t[:, :, tA, :],
                    )
                    if tB is None:
                        continue
                    wb_v = wtap[DIM:, j, :].rearrange("p (h o) -> p h o", h=2)
                    for hh in range(2):
                        nc.vector.tensor_copy(
                            wb_v[:, hh, :],
                            ps_d[hh][DIM:, tB * 32 : (tB + 1) * 32],
                        )

                def conv(img):
                    """conv, 5 matmuls per half: 3 vertical pairs
                    (0,dx)+(2,dx) on A2/w_conv6, the pair (1,0)+(1,1) on
                    A/w_conv2, and the lone (1,2) tap at K=64"""
                    xact_cur, xact2_cur = downed[img][1]
                    ps_cs = []
                    for rh in range(RH):
                        ps_c = ps_cp.tile(
                            [DIM, NHALF], F32, tag="psc", name=f"psc{img}_{rh}"
                        )
                        for dx in range(3):
                            src = xact2_cur[
                                :, rh * RROWS : rh * RROWS + RROWS, dx : dx + W
                            ]
                            nc.tensor.matmul(
                                ps_c[:],
                                wtap[:, dx, :],
                                src,
                                start=(dx == 0),
                                stop=False,
                            )
                        nc.tensor.matmul(
                            ps_c[:],
                            wtap[:, 3, :],
                            xact_cur[
                                :, rh * RROWS + 1 : rh * RROWS + 1 + RROWS, 0:W
                            ],
                            start=False,
                            stop=False,
                        )
                        nc.tensor.matmul(
                            ps_c[:],
                            wtap[:DIM, 4, :],
                            xact_cur[
                                :DIM,
                                rh * RROWS + 1 : rh * RROWS + 1 + RROWS,
                                2 : 2 + W,
                            ],
                            start=False,
                            stop=True,
                        )
                        ps_cs.append(ps_c)
                    return ps_cs

                def up_part(img, o_sb, y_act, half, alternate=False):
                    """4 up matmuls (2 c-chunks) + psum->sbuf casts."""
                    for j, (kc, rh) in enumerate(
                        (kc, rh)
                        for kc in ((0, 1) if half == 0 else (2, 3))
                        for rh in range(RH)
                    ):
                        ps_u = ps_up.tile(
                            [128, NHALF], F32, tag="psu", name=f"psu{img}_{kc}_{rh}"
                        )
                        nc.tensor.matmul(
                            ps_u[:],
                            w_up65[0:DIM, kc * 128 : (kc + 1) * 128],
                            y_act[:, rh * NHALF : (rh + 1) * NHALF],
                            start=True,
                            stop=True,
                        )
                        dst = o_sb[:, kc, rh * NHALF : (rh + 1) * NHALF]
                        if (alternate and j % 2 == 1) or (
                            not alternate and half == 1 and kc == 3 and rh == 1
                        ):
                            nc.scalar.activation(
                                dst, ps_u[:],
                                mybir.ActivationFunctionType.Identity,
                                bias=b_up_c[kc], scale=1.0,
                            )
                        else:
                            nc.vector.tensor_scalar_add(dst, ps_u[:], b_up_c[kc])

                # Steady-state block for image i (software-pipelined):
                #   PE:     up_a(i-1), conv(i), up_b(i-1), down(i+2)
                #   scalar: gelu2(i), gelu1(i+2), A2-bottom trigger(i+2)
                #   vector: casts(i-1)
                #   gpsimd: y-ones(i), pad memsets(i+2), A2-top copy(i+2)
                #   sync:   load(i+3), store(i-1), A-bottom trigger(i+2)
                # Splitting up(i-1) around conv(i) gives the vector casts a
                # conv's worth of slack before the second psum rotation, so
                # the in-order PE queue never waits on a cast.
                y_acts = {}
                o_sbs = {}
                for img in range(B_LOC):
                    if img + 3 < B_LOC:
                        xTs.append(load_x(img + 3))
                    if img >= 1:
                        o_sbs[img - 1] = outsp.tile(
                            [128, KCH, PIX], BF16, tag="o", name=f"o{img-1}"
                        )
                        up_part(img - 1, o_sbs[img - 1], y_acts[img - 1], 0)

                    ps_cs = conv(img)

                    # gelu2: quickgelu(scale*y) = Gelu_apprx_sigmoid(scale*y)
                    # straight from psum into y_act (ones row fuses up bias)
                    y_act = yactp.tile([DIM, PIX], BF16, tag="ya")
                    y_acts[img] = y_act
                    for rh in range(RH):
                        nc.scalar.activation(
                            y_act[:DIM, rh * NHALF : (rh + 1) * NHALF],
                            ps_cs[rh][:],
                            ACT_QGELU,
                            bias=0.0,
                            scale=scale_sb[:],
                        )

                    if img >= 1:
                        up_part(img - 1, o_sbs[img - 1], y_acts[img - 1], 1)
                        nc.sync.dma_start(
                            out_d[img - 1][:],
                            o_sbs[img - 1][:].rearrange("p k n -> p (k n)"),
                        )

                    if img + 2 < B_LOC:
                        xacts.append(make_xacts(img + 2))
                        issue_down(img + 2)

                # drain: last image's up with casts split across vector and
                # scalar (both idle now), store in two halves so the first
                # starts before the second is cast
                li = B_LOC - 1
                o_sb = outsp.tile([128, KCH, PIX], BF16, tag="o")
                up_part(li, o_sb, y_acts[li], 0, alternate=True)
                nc.sync.dma_start(
                    out_d[li][:, : 2 * PIX],
                    o_sb[:, 0:2, :].rearrange("p k n -> p (k n)"),
                )
                up_part(li, o_sb, y_acts[li], 1, alternate=True)
                nc.sync.dma_start(
                    out_d[li][:, 2 * PIX :],
                    o_sb[:, 2:4, :].rearrange("p k n -> p (k n)"),
                )

    nc.compile()
    _CACHE["nc"] = nc
    return nc


def _make_in_maps(inputs):
    bf16 = ml_dtypes.bfloat16
    x = np.ascontiguousarray(inputs["x"], dtype=np.float32)

    # ---- packed bf16 consts ----
    cpk = np.zeros((128, CPACK_W), dtype=bf16)
    wd = np.asarray(inputs["w_down"], np.float32).astype(bf16)
    t = wd.reshape(KCH, 128, DIM).transpose(1, 0, 2)       # [p, k, d]
    cpk[:, CP_WDOWN : CP_WDOWN + 512] = np.concatenate(
        [t, t], axis=2
    ).reshape(128, 512)
    cpk[0:DIM, CP_WUP : CP_WUP + 512] = np.asarray(
        inputs["w_up"], np.float32
    ).astype(bf16)
    cpk[DIM, CP_WUP : CP_WUP + 512] = np.asarray(
        inputs["b_up"], np.float32
    ).astype(bf16)
    emb = np.asarray(inputs["layer_emb"], np.float32).astype(bf16)
    cpk[0:EMB, CP_T2 + 64] = emb
    cpk[EMB:128, CP_T2 + 128] = emb
    bh = np.asarray(inputs["b_hyper"], np.float32).reshape(DIM, DIM, 9)
    b_ot = bh.transpose(1, 0, 2).astype(bf16)              # [i, o, t]
    cpk[0:DIM, CP_BPREP : CP_BPREP + NHYP] = (
        b_ot[:, :32].transpose(0, 2, 1).reshape(DIM, NHYP)   # [i, (t, ol)]
    )
    cpk[DIM:, CP_BPREP : CP_BPREP + NHYP] = (
        b_ot[:, 32:].transpose(0, 2, 1).reshape(DIM, NHYP)
    )

    sel = np.zeros((128, 256), dtype=bf16)
    for m in range(DIM):
        sel[m + DIM, m] = 1.0           # shift-up-64:  out row m <- row m+64
    for p in range(DIM):
        sel[p, 128 + DIM + p] = 1.0     # shift-dn-64:  out row p+64 <- row p
    cpk[:, CP_SEL:] = sel

    cf = np.zeros((128, 6), np.float32)
    bd = np.asarray(inputs["b_down"], np.float32)
    cf[0:DIM, 0] = bd
    cf[DIM:, 0] = bd
    cf[0:DIM, 1] = np.asarray(inputs["scale"], np.float32)
    bu = np.asarray(inputs["b_up"], np.float32)
    for kc in range(KCH):
        cf[:, 2 + kc] = bu[kc * 128 : (kc + 1) * 128]

    # ---- packed hypernet: [128, i, ol, t]; rows 0-63 = o<32 block ----
    wh = np.asarray(inputs["w_hyper"], np.float32).astype(bf16)
    wh = wh.reshape(EMB, DIM, DIM, 9)                      # [e, o, i, t]
    top = wh[:, :32].transpose(0, 2, 3, 1)                 # [e, i, t, ol]
    bot = wh[:, 32:].transpose(0, 2, 3, 1)
    whp = np.ascontiguousarray(
        np.concatenate([top, bot], axis=0).reshape(128, DIM * NHYP)
    )

    shared = {"cpack": cpk, "cf32": cf, "w_hyper": whp}
    in_maps = []
    for c in range(NCORES):
        xc = x[c * B_LOC : (c + 1) * B_LOC].reshape(B_LOC, PIX, KCH, 128)
        xt = np.ascontiguousarray(xc.transpose(0, 3, 2, 1)).astype(bf16)
        in_maps.append({"x": xt.reshape(B_LOC, 128, KCH * PIX), **shared})
    return in_maps


def _untranspose_out(res):
    outs = []
    for c in range(NCORES):
        o = np.asarray(res.results[c]["out"]).reshape(B_LOC, 128, KCH, PIX)
        o = o.transpose(0, 3, 2, 1).astype(np.float32)  # [img, pix, kc, p]
        outs.append(o.reshape(B_LOC, H, W, C))
    return np.concatenate(outs, axis=0)


def kernel(**inputs) -> np.ndarray:
    nc = build_kernel()
    in_maps = _make_in_maps(inputs)
    res = run_bass_kernel_spmd(nc, in_maps, core_ids=list(range(NCORES)))
    return _untranspose_out(res)


def run_traced(inputs, **kw):
    """For test.py: run with tracing to get HW exec time."""
    nc = build_kernel()
    in_maps = _make_in_maps(inputs)
    return run_bass_kernel_spmd(
        nc, in_maps, core_ids=list(range(NCORES)), trace=True, **kw
    )


# revision 27
# speedup vs baseline: 1.0583x; 1.0227x over previous
"""Trainium2 Bass kernel: Convpass adapter with hypernet-generated 3x3 conv.

Per core (data-parallel over batch, 8 images/core):
  hypernet: conv_w = emb @ w_hyper + b_hyper, via the diag-window matmul
            trick with both o-halves packed on 128 partitions (64 matmuls
            of N=288). The 4.7MB bf16 w_hyper streams in 8 DMA chunks with
            matmuls chasing the chunks; w_conv2 is built in two 32-row
            groups overlapped with the stream.
  down:     xT[128c,4k,784] @ [w_down|w_down] -> psum [128, 392] per half
            (x arrives pre-transposed bf16 from the host)
  gelu1:    quickgelu(x+b) as ONE scalar activation per half from psum
            rows 0-63 into the padded A-top region; the three other
            conv-packing copies are derived from A-top:
              A-bottom  = A-top shifted by 1 flat element  (packs dx-pairs)
              A2-top    = A-top verbatim                   (gpsimd copy)
              A2-bottom = A-top shifted by 60 flat elements (packs dy-pairs)
            The two shifts are single CONTIGUOUS whole-image SBUF->SBUF
            DMAs (64 fat descriptors each) because a row/col shift in the
            padded [30,30] layout is a pure flat-offset shift; the pad
            columns wrap into each other and carry zeros. The previous
            per-(row,half) sliced DMA form generated 832 56-byte
            descriptors per call (13K packets/kernel) which saturated the
            DMA engines and starved the PE into HAM-throttled 1.2GHz.
  conv:     3x3 as 3 K=128 matmuls (dy-pairs on A2/w_conv6) + 1 K=128
            (dx-pair on A/w_conv2) + 1 K=64 (lone tap)
  gelu2:    quickgelu(scale*y) as one activation per half into y_act
  up:       out^T[128c,392] = w_up65[:,cslice].T @ y_act  (stationary w_up,
            ones-row fused bias); stored transposed bf16, host untransposes.

Engine budget per image (targeting warm-PE 4.2us/image):
  PE 26 matmuls; scalar 4 acts; vector 8 psum->sbuf casts; gpsimd pad
  memsets + A2-top copy + y_act ones row; sync queue all DMA triggers
  except A2-bottom (scalar).

All small constants are assembled host-side in bf16 and loaded as one
contiguous tensor over the fast hardware DGE ring. The image loop is
software-pipelined for the in-order PE queue: tensor order is conv(i),
down(i+2), up(i), with gelu1(i+2) issued right after down(i+2).
"""

import os

import numpy as np
import ml_dtypes

import concourse.bass as bass
import concourse.mybir as mybir
import concourse.tile as tile
from concourse import bacc
from concourse.bass_utils import run_bass_kernel_spmd

# Problem shapes (hardcoded per contract).
B, H, W, C = 64, 28, 28, 512
DIM, EMB = 64, 64
NCORES = 8
B_LOC = B // NCORES            # 8 images per core
PIX = H * W                    # 784 pixels per image
PW = W + 2                     # 30 padded width
PAD = PW * (H + 2)             # 900 padded pixels per image
RH = 2                         # row-halves per image
RROWS = H // RH                # 14 rows per half
NHALF = RROWS * W              # 392 pixels per half-tile
KCH = C // 128                 # 4 contraction chunks of 128 channels
JTOT = DIM * DIM * 9           # 36864 hypernet outputs
NHYP = 32 * 9                  # 288 = free size of packed hypernet matmuls

NCHUNK = 8                     # w_hyper streaming DMA chunks
ICH = DIM // NCHUNK            # 8 i-rows per DMA chunk
IGRP = 32                      # i-rows per compute/build group (32-aligned)

# packed-const column offsets (bf16 [128, CPACK_W])
CP_WDOWN = 0                   # [128, 512]  w_down duplicated, (k m) layout
CP_WUP = 512                   # [65, 512]   w_up with bias row 64
CP_T2 = 1024                   # [128, 192]  hypernet lhsT window tensor
CP_BPREP = 1216                # [128, 288]  conv bias, psum-row layout
CP_SEL = 1504                  # [128, 128] shift-up-64 + [64, 128] shift-dn-64
CPACK_W = 1760

F32 = mybir.dt.float32
BF16 = mybir.dt.bfloat16
GELU_A = 1.702
# CoreSim doesn't implement Gelu_apprx_sigmoid; substitute Sigmoid for
# structure-only sim runs (numerics then checked on HW via --randup).
ACT_QGELU = (
    mybir.ActivationFunctionType.Sigmoid
    if os.environ.get("KERNEL_DEBUG_SIM_ACT") == "1"
    else mybir.ActivationFunctionType.Gelu_apprx_sigmoid
)

_CACHE = {}


def build_kernel():
    if "nc" in _CACHE:
        return _CACHE["nc"]

    nc = bacc.Bacc("TRN2", target_bir_lowering=False, debug=False)

    x_d = nc.dram_tensor("x", [B_LOC, 128, KCH * PIX], BF16, kind="ExternalInput")
    cpk_d = nc.dram_tensor("cpack", [128, CPACK_W], BF16, kind="ExternalInput")
    cf_d = nc.dram_tensor("cf32", [128, 6], F32, kind="ExternalInput")
    # host-packed hypernet: [128, i, ol, t]; rows 0-63 = o<32, 64-127 = o>=32
    wh_d = nc.dram_tensor("w_hyper", [128, DIM * NHYP], BF16, kind="ExternalInput")
    out_d = nc.dram_tensor("out", [B_LOC, 128, KCH * PIX], BF16, kind="ExternalOutput")

    with tile.TileContext(nc) as tc:
        with tc.tile_pool(name="consts", bufs=1) as consts:
            # ---- constants: ONE cpack DMA (0.38MB, first on sync) + cf32;
            # everything else is a view into cpk_sb ----
            cpk_sb = consts.tile([128, CPACK_W], BF16)
            nc.sync.dma_start(cpk_sb[:], cpk_d[:])
            cf_sb = consts.tile([128, 6], F32)
            nc.sync.dma_start(cf_sb[:], cf_d[:])
            t2 = cpk_sb[:, CP_T2 : CP_T2 + 192]
            w_down2 = cpk_sb[:, CP_WDOWN : CP_WDOWN + 512].rearrange(
                "p (k m) -> p k m", k=KCH
            )
            b_prep2 = cpk_sb[:, CP_BPREP : CP_BPREP + NHYP]
            w_up65 = cpk_sb[:, CP_WUP : CP_WUP + 512]
            b_down2 = cf_sb[:, 0:1]
            scale_sb = cf_sb[0:DIM, 1:2]
            b_up_c = [cf_sb[:, 2 + kc : 3 + kc] for kc in range(KCH)]
            # 0/1 selector stationaries: matmuls route partitions, so the
            # hypernet-build partition moves never touch a DMA ring (ring
            # transfers were landing ~9us late behind the x-image backlog)
            sel_up = cpk_sb[:, CP_SEL : CP_SEL + 128]
            sel_dn = cpk_sb[0:DIM, CP_SEL + 128 : CP_SEL + 256]

            # PE warm-up: the HAM clock gate holds the PE at 1.2GHz until it
            # sees ~3.4us of sustained busy. Burn that window on dummy
            # matmuls over a scratch tile while the prologue DMAs stream, so
            # the hypernet and image matmuls all run at 2.4GHz. The two tiny
            # dummy activations preload the Gelu/Copy LUTs so the 1.3us
            # ACT_TABLE_LOADs don't land in the act critical path later.
            scratch = consts.tile([128, 128], BF16)
            nc.vector.memset(scratch[:], 0.25)
            dum = consts.tile([1, 16], BF16)
            nc.scalar.activation(
                dum[:], scratch[0:1, 0:16], ACT_QGELU, bias=0.0, scale=1.0
            )
            nc.scalar.copy(dum[:], scratch[0:1, 0:16])

            # w_conv2 top rows: W[i, (h, t, o32)] assembled from the
            # hypernet psum; only rows 0-63 are ever written/read (the
            # tap-pair stationaries live in wtap below)
            w_conv2 = consts.tile([128, DIM * 9], BF16)

            # ---- main pools ----
            with (
                tc.tile_pool(name="whpool", bufs=1) as whpool,
                tc.tile_pool(name="xin", bufs=4) as xin,
                tc.tile_pool(name="xact", bufs=4) as xactp,
                tc.tile_pool(name="xact2", bufs=4) as xact2p,
                tc.tile_pool(name="yact", bufs=3) as yactp,
                tc.tile_pool(name="tmp", bufs=6) as tmpp,
                tc.tile_pool(name="outs", bufs=2) as outsp,
                tc.tile_pool(name="ps_s", bufs=2, space="PSUM") as ps_sp,
                tc.tile_pool(name="ps_c", bufs=2, space="PSUM") as ps_cp,
                tc.tile_pool(name="ps_u", bufs=3, space="PSUM") as ps_up,
                tc.tile_pool(name="ps_h", bufs=1, space="PSUM") as ps_hp,
            ):
                # ---- prologue ----
                # warm-up burn (~3.6us of PE busy) in a dedicated psum bank
                # so the WAW chain of later heartbeats never blocks the
                # rotating up-proj psum tiles
                ps_w = ps_hp.tile([128, 128], F32, tag="hb", name="warm")
                for _ in range(34):
                    nc.tensor.matmul(
                        ps_w[:], scratch[:], scratch[:],
                        start=True, stop=True,
                    )

                def heartbeat(rhs64):
                    """Tiny matmul whose rhs is freshly-DMAed data: fires
                    exactly when that transfer lands, keeping the HAM
                    activity window non-idle through DMA-bound stretches
                    (an idle >3.4us re-throttles the PE to 1.2GHz, and it
                    has been observed stuck cold for 25us+ afterwards)."""
                    nc.tensor.matmul(
                        ps_w[0:64, 0:64], scratch[:, 0:64], rhs64,
                        start=True, stop=True,
                    )

                # one tile PER w_hyper chunk: tile-granular dependency
                # tracking would stall the first hypernet matmul until the
                # last chunk landed if this were a single tile
                def load_x(img, eng=None):
                    xT = xin.tile([128, KCH, PIX], BF16, tag="x", name=f"x{img}")
                    (eng or nc.sync).dma_start(
                        xT[:].rearrange("p k n -> p (k n)"), x_d[img]
                    )
                    heartbeat(xT[:, 0, 0:64])
                    return xT

                # queue split (each HWDGE data ring drains in FIFO
                # order, and triggers enter a ring the moment their deps
                # are ready -- so big loads must not be able to jump in
                # front of latency-critical small transfers):
                #   scalar ring: ALL wh chunks (drained ~27us), then only
                #     the small shift/build transfers (land promptly)
                #   sync ring:   cpack, cf32, x0.., stores (big streams)
                xTs = [load_x(0)]
                wh_chunks = []
                for q in range(NCHUNK):
                    cw = ICH * NHYP
                    t = whpool.tile([128, cw], BF16, tag=f"wh{q}")
                    eng = nc.scalar if q % 2 == 0 else nc.sync
                    eng.dma_start(t[:], wh_d[:, q * cw : (q + 1) * cw])
                    heartbeat(t[:, 0:64])
                    wh_chunks.append(t)
                xTs.append(load_x(1))
                xTs.append(load_x(2))

                def make_xacts(img):
                    """A/A2 padded buffers; one full memset of A-top zeroes
                    the pads (strided pad-only memsets measured slower on
                    gpsimd than one contiguous sweep); the act overwrites
                    the interior, the derived copies carry pads along."""
                    x_act = xactp.tile([128, PAD], BF16, tag="xa", name=f"xa{img}")
                    nc.gpsimd.memset(x_act[0:DIM, :], 0.0)
                    x_act2 = xact2p.tile([128, PAD], BF16, tag="xb", name=f"xb{img}")
                    return (x_act, x_act2)

                xacts = [make_xacts(0), make_xacts(1)]

                def wh_slice(il):
                    """rhs [128, 288] for hypernet row il, from its chunk."""
                    t = wh_chunks[il // ICH]
                    j = il % ICH
                    return t[:, j * NHYP : (j + 1) * NHYP]

                def down(img, xT):
                    """down-proj matmuls -> 2 psum tiles [128, 392]"""
                    ps_ds = [
                        ps_sp.tile([128, NHALF], F32, tag="pss", name=f"psd{img}_{rh}")
                        for rh in range(RH)
                    ]
                    for k in range(KCH):
                        for rh in range(RH):
                            nc.tensor.matmul(
                                ps_ds[rh][:],
                                w_down2[:, k, :],
                                xT[:, k, rh * NHALF : (rh + 1) * NHALF],
                                start=(k == 0),
                                stop=(k == KCH - 1),
                            )
                    return ps_ds

                def gelu1(img, ps_ds, x_act, x_act2):
                    """quickgelu(x+b) = Gelu_apprx_sigmoid(1.0*x + b) from
                    psum rows 0-63 into A-top (padded rows 1..28, cols
                    1..28)."""
                    x_act_v = x_act[:].rearrange("d (r c) -> d r c", c=PW)
                    for rh in range(RH):
                        ps_v = ps_ds[rh][:].rearrange("d (r c) -> d r c", c=W)
                        rows = slice(1 + rh * RROWS, 1 + (rh + 1) * RROWS)
                        nc.scalar.activation(
                            x_act_v[:DIM, rows, 1 : 1 + W],
                            ps_v[:DIM],
                            ACT_QGELU,
                            bias=b_down2[:DIM],
                            scale=1.0,
                        )
                    return (x_act_v, x_act2[:].rearrange("d (r c) -> d r c", c=PW))

                def shifts(img, x_act, x_act2):
                    """Derive the three other conv-packing copies from
                    A-top. conv(img) is ~1.5 image-blocks away, so these
                    can trail the gelu2/up work in each queue."""
                    # A-bottom: act shifted one flat element (one padded
                    # col); pad cols wrap into each other and carry zeros
                    nc.scalar.dma_start(
                        x_act[DIM:, 0 : PAD - 1], x_act[:DIM, 1:PAD]
                    )
                    # A2-top duplicates A-top verbatim (pads included);
                    # vector: gpsimd measured 3.2us for this copy vs 0.9
                    nc.vector.tensor_copy(x_act2[:DIM, :], x_act[:DIM, :])
                    # A2-bottom: act shifted one padded row-pair (2 rows =
                    # 60 flat elements)
                    nc.scalar.dma_start(
                        x_act2[DIM:, 0 : PAD - 2 * PW], x_act[:DIM, 2 * PW : PAD]
                    )

                # Prologue: downs go AFTER the first hypernet group in the
                # in-order PE queue (a stalled down matmul ahead of the
                # hypernet delays conv0 behind it).
                downed = []

                def issue_down(img, with_shifts=True):
                    ps = down(img, xTs[img])
                    xa = gelu1(img, ps, *xacts[img])
                    if with_shifts:
                        shifts(img, *xacts[img])
                    downed.append((ps, xa))

                # full hypernet: matmuls chase the streaming w_hyper DMA
                # chunk by chunk (region-level deps); psum/build work in two
                # 32-row groups (engine partition slices need 32 alignment).
                # Group g's psum rows [32g,32g+32) = W[i, o<32], rows
                # [64+32g, ..) = W[i, o>=32].
                t_b = tmpp.tile([128, NHYP], BF16, tag="t", name="t_b")
                for g in range(DIM // IGRP):
                    ps_q = ps_up.tile([128, NHYP], F32, tag="psu", name=f"hyp{g}")
                    for il in range(g * IGRP, (g + 1) * IGRP):
                        nc.tensor.matmul(
                            ps_q[:],
                            t2[:, 64 - il : 192 - il],
                            wh_slice(il),
                            start=(il % IGRP == 0),
                            stop=(il % IGRP == IGRP - 1),
                        )
                    if g == 0:
                        # two images' downs fill the w_hyper chunk-wait gaps
                        # between the hypernet groups
                        issue_down(0)
                        issue_down(1)
                    rt = slice(g * IGRP, (g + 1) * IGRP)
                    rb = slice(DIM + g * IGRP, DIM + (g + 1) * IGRP)
                    nc.vector.tensor_tensor(
                        w_conv2[rt, :NHYP], ps_q[rt, :], b_prep2[rt, :],
                        mybir.AluOpType.add,
                    )
                    nc.vector.tensor_tensor(
                        t_b[rb, :], ps_q[rb, :], b_prep2[rb, :],
                        mybir.AluOpType.add,
                    )
                # o>=32 block: PE routes t_b partitions 64+r -> r (matmul
                # operands must start at partition 0/32/64, so one K=64
                # routing matmul covers both groups), vector drains psum
                # into w_conv2 cols 288:576
                ps_m = ps_up.tile([128, NHYP], F32, tag="psu", name="mv")
                nc.tensor.matmul(
                    ps_m[:], sel_up[DIM:, :], t_b[DIM:, :],
                    start=True, stop=True,
                )
                nc.vector.tensor_copy(w_conv2[0:DIM, NHYP:], ps_m[0:DIM, :])
                # PE routes w_conv2 top rows r -> 64+r into psum (2 col
                # halves), then the 5 tap-pair stationaries are assembled
                # straight from SBUF-top/psum-bottom: wtap block rows 0-63 =
                # tap tA, rows 64-127 = tap tB (the pair contracted by one
                # K=128 matmul). 0..2 = (0,dx)+(2,dx), 3 = (1,0)+(1,1),
                # 4 = lone (1,2) (top half only). matmul lhsT APs may only
                # have ONE free dim, so these are contiguous [128, 64].
                ps_d = []
                for hh in range(2):
                    p = ps_up.tile([128, NHYP], F32, tag="psu", name=f"dn{hh}")
                    nc.tensor.matmul(
                        p[:], sel_dn[:, :],
                        w_conv2[0:DIM, hh * NHYP : (hh + 1) * NHYP],
                        start=True, stop=True,
                    )
                    ps_d.append(p)
                w2t = w_conv2[0:DIM, :].rearrange("p (h t o) -> p h t o", h=2, t=9)
                wtap = consts.tile([128, 5, 64], BF16)
                # lone-tap block: bottom 64 rows are ZERO so the matmul can
                # contract K=128 (K<=65 matmuls measured ~1.7x slower per
                # column than K=128)
                nc.vector.memset(wtap[DIM:, 4, :], 0.0)
                for j, (tA, tB) in enumerate(
                    [(0, 6), (1, 7), (2, 8), (3, 4), (5, None)]
                ):
                    nc.vector.tensor_copy(
                        wtap[0:DIM, j, :].rearrange("p (h o) -> p h o", h=2),
                        w2t[:, :, tA, :],
                    )
                    if tB is None:
                        continue
                    wb_v = wtap[DIM:, j, :].rearrange("p (h o) -> p h o", h=2)
                    for hh in range(2):
                        nc.vector.tensor_copy(
                            wb_v[:, hh, :],
                            ps_d[hh][DIM:, tB * 32 : (tB + 1) * 32],
                        )

                def conv(img):
                    """conv, 5 matmuls per half: 3 vertical pairs
                    (0,dx)+(2,dx) on A2/w_conv6, the pair (1,0)+(1,1) on
                    A/w_conv2, and the lone (1,2) tap at K=64"""
                    xact_cur, xact2_cur = downed[img][1]
                    ps_cs = []
                    for rh in range(RH):
                        ps_c = ps_cp.tile(
                            [DIM, NHALF], F32, tag="psc", name=f"psc{img}_{rh}"
                        )
                        for dx in range(3):
                            src = xact2_cur[
                                :, rh * RROWS : rh * RROWS + RROWS, dx : dx + W
                            ]
                            nc.tensor.matmul(
                                ps_c[:],
                                wtap[:, dx, :],
                                src,
                                start=(dx == 0),
                                stop=False,
                            )
                        nc.tensor.matmul(
                            ps_c[:],
                            wtap[:, 3, :],
                            xact_cur[
                                :, rh * RROWS + 1 : rh * RROWS + 1 + RROWS, 0:W
                            ],
                            start=False,
                            stop=False,
                        )
                        nc.tensor.matmul(
                            ps_c[:],
                            wtap[:, 4, :],
                            xact_cur[
                                :,
                                rh * RROWS + 1 : rh * RROWS + 1 + RROWS,
                                2 : 2 + W,
                            ],
                            start=False,
                            stop=True,
                        )
                        ps_cs.append(ps_c)
                    return ps_cs

                def up_part(img, o_sb, y_act, half, alternate=False):
                    """4 up matmuls (2 c-chunks) + psum->sbuf casts."""
                    for j, (kc, rh) in enumerate(
                        (kc, rh)
                        for kc in ((0, 1) if half == 0 else (2, 3))
                        for rh in range(RH)
                    ):
                        ps_u = ps_up.tile(
                            [128, NHALF], F32, tag="psu", name=f"psu{img}_{kc}_{rh}"
                        )
                        nc.tensor.matmul(
                            ps_u[:],
                            w_up65[:, kc * 128 : (kc + 1) * 128],
                            y_act[:, rh * NHALF : (rh + 1) * NHALF],
                            start=True,
                            stop=True,
                        )
                        dst = o_sb[:, kc, rh * NHALF : (rh + 1) * NHALF]
                        if (alternate and j % 2 == 1) or (
                            not alternate and half == 1 and kc == 3 and rh == 1
                        ):
                            nc.scalar.activation(
                                dst, ps_u[:],
                                mybir.ActivationFunctionType.Identity,
                                bias=b_up_c[kc], scale=1.0,
                            )
                        else:
                            nc.vector.tensor_scalar_add(dst, ps_u[:], b_up_c[kc])

                # Steady-state block for image i (software-pipelined):
                #   PE:     up_a(i-1), conv(i), up_b(i-1), down(i+2)
                #   scalar: gelu2(i), gelu1(i+2), A2-bottom trigger(i+2)
                #   vector: casts(i-1)
                #   gpsimd: y-ones(i), pad memsets(i+2), A2-top copy(i+2)
                #   sync:   load(i+3), store(i-1), A-bottom trigger(i+2)
                # Splitting up(i-1) around conv(i) gives the vector casts a
                # conv's worth of slack before the second psum rotation, so
                # the in-order PE queue never waits on a cast.
                y_acts = {}
                o_sbs = {}
                for img in range(B_LOC):
                    if img + 3 < B_LOC:
                        xTs.append(load_x(img + 3))
                    if img >= 1:
                        o_sbs[img - 1] = outsp.tile(
                            [128, KCH, PIX], BF16, tag="o", name=f"o{img-1}"
                        )
                        up_part(img - 1, o_sbs[img - 1], y_acts[img - 1], 0)

                    ps_cs = conv(img)

                    # gelu2: quickgelu(scale*y) = Gelu_apprx_sigmoid(scale*y)
                    # straight from psum into y_act (ones row fuses up bias)
                    y_act = yactp.tile([128, PIX], BF16, tag="ya")
                    y_acts[img] = y_act
                    nc.gpsimd.memset(y_act[DIM:, :], 0.0)
                    for rh in range(RH):
                        nc.scalar.activation(
                            y_act[:DIM, rh * NHALF : (rh + 1) * NHALF],
                            ps_cs[rh][:],
                            ACT_QGELU,
                            bias=0.0,
                            scale=scale_sb[:],
                        )

                    if img >= 1:
                        up_part(img - 1, o_sbs[img - 1], y_acts[img - 1], 1)
                        nc.sync.dma_start(
                            out_d[img - 1][:],
                            o_sbs[img - 1][:].rearrange("p k n -> p (k n)"),
                        )

                    if img + 2 < B_LOC:
                        xacts.append(make_xacts(img + 2))
                        issue_down(img + 2)

                # drain: last image's up with casts split across vector and
                # scalar (both idle now), store in two halves so the first
                # starts before the second is cast
                li = B_LOC - 1
                o_sb = outsp.tile([128, KCH, PIX], BF16, tag="o")
                up_part(li, o_sb, y_acts[li], 0, alternate=True)
                nc.sync.dma_start(
                    out_d[li][:, : 2 * PIX],
                    o_sb[:, 0:2, :].rearrange("p k n -> p (k n)"),
                )
                up_part(li, o_sb, y_acts[li], 1, alternate=True)
                nc.sync.dma_start(
                    out_d[li][:, 2 * PIX :],
                    o_sb[:, 2:4, :].rearrange("p k n -> p (k n)"),
                )

    nc.compile()
    _CACHE["nc"] = nc
    return nc


def _make_in_maps(inputs):
    bf16 = ml_dtypes.bfloat16
    x = np.ascontiguousarray(inputs["x"], dtype=np.float32)

    # ---- packed bf16 consts ----
    cpk = np.zeros((128, CPACK_W), dtype=bf16)
    wd = np.asarray(inputs["w_down"], np.float32).astype(bf16)
    t = wd.reshape(KCH, 128, DIM).transpose(1, 0, 2)       # [p, k, d]
    cpk[:, CP_WDOWN : CP_WDOWN + 512] = np.concatenate(
        [t, t], axis=2
    ).reshape(128, 512)
    cpk[0:DIM, CP_WUP : CP_WUP + 512] = np.asarray(
        inputs["w_up"], np.float32
    ).astype(bf16)
    cpk[DIM, CP_WUP : CP_WUP + 512] = np.asarray(
        inputs["b_up"], np.float32
    ).astype(bf16)
    emb = np.asarray(inputs["layer_emb"], np.float32).astype(bf16)
    cpk[0:EMB, CP_T2 + 64] = emb
    cpk[EMB:128, CP_T2 + 128] = emb
    bh = np.asarray(inputs["b_hyper"], np.float32).reshape(DIM, DIM, 9)
    b_ot = bh.transpose(1, 0, 2).astype(bf16)              # [i, o, t]
    cpk[0:DIM, CP_BPREP : CP_BPREP + NHYP] = (
        b_ot[:, :32].transpose(0, 2, 1).reshape(DIM, NHYP)   # [i, (t, ol)]
    )
    cpk[DIM:, CP_BPREP : CP_BPREP + NHYP] = (
        b_ot[:, 32:].transpose(0, 2, 1).reshape(DIM, NHYP)
    )

    sel = np.zeros((128, 256), dtype=bf16)
    for m in range(DIM):
        sel[m + DIM, m] = 1.0           # shift-up-64:  out row m <- row m+64
    for p in range(DIM):
        sel[p, 128 + DIM + p] = 1.0     # shift-dn-64:  out row p+64 <- row p
    cpk[:, CP_SEL:] = sel

    cf = np.zeros((128, 6), np.float32)
    bd = np.asarray(inputs["b_down"], np.float32)
    cf[0:DIM, 0] = bd
    cf[DIM:, 0] = bd
    cf[0:DIM, 1] = np.asarray(inputs["scale"], np.float32)
    bu = np.asarray(inputs["b_up"], np.float32)
    for kc in range(KCH):
        cf[:, 2 + kc] = bu[kc * 128 : (kc + 1) * 128]

    # ---- packed hypernet: [128, i, ol, t]; rows 0-63 = o<32 block ----
    wh = np.asarray(inputs["w_hyper"], np.float32).astype(bf16)
    wh = wh.reshape(EMB, DIM, DIM, 9)                      # [e, o, i, t]
    top = wh[:, :32].transpose(0, 2, 3, 1)                 # [e, i, t, ol]
    bot = wh[:, 32:].transpose(0, 2, 3, 1)
    whp = np.ascontiguousarray(
        np.concatenate([top, bot], axis=0).reshape(128, DIM * NHYP)
    )

    shared = {"cpack": cpk, "cf32": cf, "w_hyper": whp}
    in_maps = []
    for c in range(NCORES):
        xc = x[c * B_LOC : (c + 1) * B_LOC].reshape(B_LOC, PIX, KCH, 128)
        xt = np.ascontiguousarray(xc.transpose(0, 3, 2, 1)).astype(bf16)
        in_maps.append({"x": xt.reshape(B_LOC, 128, KCH * PIX), **shared})
    return in_maps


def _untranspose_out(res):
    outs = []
    for c in range(NCORES):
        o = np.asarray(res.results[c]["out"]).reshape(B_LOC, 128, KCH, PIX)
        o = o.transpose(0, 3, 2, 1).astype(np.float32)  # [img, pix, kc, p]
        outs.append(o.reshape(B_LOC, H, W, C))
    return np.concatenate(outs, axis=0)


def kernel(**inputs) -> np.ndarray:
    nc = build_kernel()
    in_maps = _make_in_maps(inputs)
    res = run_bass_kernel_spmd(nc, in_maps, core_ids=list(range(NCORES)))
    return _untranspose_out(res)


def run_traced(inputs, **kw):
    """For test.py: run with tracing to get HW exec time."""
    nc = build_kernel()
    in_maps = _make_in_maps(inputs)
    return run_bass_kernel_spmd(
        nc, in_maps, core_ids=list(range(NCORES)), trace=True, **kw
    )


# revision 28
# speedup vs baseline: 1.1283x; 1.0661x over previous
"""Trainium2 Bass kernel: Convpass adapter with hypernet-generated 3x3 conv.

Per core (data-parallel over batch, 8 images/core):
  hypernet: conv_w = emb @ w_hyper + b_hyper via the diag-window matmul
            trick (both o-halves packed on 128 partitions; 64 matmuls of
            N=288 chase the 4.7MB bf16 w_hyper stream, chunks alternating
            between the two HWDGE rings). The conv stationaries are then
            assembled WITHOUT touching a DMA ring: 0/1 selector matmuls
            route partitions (t_b rows 64+r -> r for the o>=32 block, top
            rows r -> 64+r for the tap-shifted pair halves) and vector
            drains psum into five contiguous [128, 64] tap-pair blocks
            (wtap). Ring transfers for these used to land ~9us late behind
            the streaming x-image backlog and gated the first conv.
  down:     xT[128c,4k,784] @ [w_down|w_down] -> psum [128, 392] per half
  gelu1:    quickgelu(x+b) as ONE activation per half from psum rows 0-63
            into padded A-top; the three other conv-packing copies derive
            from A-top: A-bottom = flat shift by 1 (one padded col),
            A2-top = verbatim vector copy, A2-bottom = flat shift by 60
            (two padded rows). The shifts are single CONTIGUOUS
            whole-image SBUF->SBUF DMAs (64 fat descriptors, scalar ring;
            the original per-(row,half) sliced form generated 13K 56-byte
            packets per kernel and starved the PE into HAM throttling).
  conv:     3x3 as 5 matmuls per half: 3 K=128 dy-pairs (0,dx)+(2,dx) on
            A2, 1 K=128 dx-pair (1,0)+(1,1) on A, and the lone (1,2) tap
            padded to K=128 with a zeroed weight block (K<=65 matmuls
            measured ~1.7x slower per column than K=128).
  gelu2:    quickgelu(scale*y) into y_act[0:64]; rows 64-127 memset 0 so
            the up-proj also contracts K=128 against w_up rows 65-127
            which are zero in the packed consts.
  up:       out^T[c,392] = w_up.T @ y_act per c-chunk, K=128; b_up is
            added during the psum->sbuf casts (vector tensor_scalar_add,
            one per image on scalar as activation Identity with AP bias).
            Stored transposed bf16; the host untransposes.

Scheduling (priority-heap tile scheduler, per-engine ready-first):
  - 34 warm-up matmuls + per-DMA-arrival heartbeat matmuls keep the HAM
    activity window alive: one PE idle >3.4us re-throttles the PE to
    1.2GHz and it has been observed stuck cold for 20-30us afterwards.
    (With all 8 cores active the chip ALSO duty-cycles the PE clock
    ~50%, which is the dominant remaining limiter and run-to-run noise.)
  - steady-state block for image i:
      PE:     up_a(i-1), conv(i), up_b(i-1), down(i+2)
      scalar: gelu2(i), gelu1(i+2), shift triggers(i+2)
      vector: biased casts(i-1), A2-top copy(i+2)
      gpsimd: pad memset + y-pad memset, heartbeats
      sync ring:   x loads + output stores (big streams only)
      scalar ring: wh chunks (prologue) + shift DMAs (small, prompt)
    Splitting up(i-1) around conv(i) gives the casts a conv's worth of
    slack before their psum banks rotate, so the in-order PE stream
    never waits on a cast.
All small constants are packed host-side into one bf16 tensor and loaded
as a single HWDGE transfer (plus a tiny fp32 one); x arrives
pre-transposed bf16 from the host.
"""
import os

import numpy as np
import ml_dtypes

import concourse.bass as bass
import concourse.mybir as mybir
import concourse.tile as tile
from concourse import bacc
from concourse.bass_utils import run_bass_kernel_spmd

# Problem shapes (hardcoded per contract).
B, H, W, C = 64, 28, 28, 512
DIM, EMB = 64, 64
NCORES = 8
B_LOC = B // NCORES            # 8 images per core
PIX = H * W                    # 784 pixels per image
PW = W + 2                     # 30 padded width
PAD = PW * (H + 2)             # 900 padded pixels per image
RH = 2                         # row-halves per image
RROWS = H // RH                # 14 rows per half
NHALF = RROWS * W              # 392 pixels per half-tile
KCH = C // 128                 # 4 contraction chunks of 128 channels
JTOT = DIM * DIM * 9           # 36864 hypernet outputs
NHYP = 32 * 9                  # 288 = free size of packed hypernet matmuls

NCHUNK = 8                     # w_hyper streaming DMA chunks
ICH = DIM // NCHUNK            # 8 i-rows per DMA chunk
IGRP = 32                      # i-rows per compute/build group (32-aligned)

# packed-const column offsets (bf16 [128, CPACK_W])
CP_WDOWN = 0                   # [128, 512]  w_down duplicated, (k m) layout
CP_WUP = 512                   # [65, 512]   w_up with bias row 64
CP_T2 = 1024                   # [128, 192]  hypernet lhsT window tensor
CP_BPREP = 1216                # [128, 288]  conv bias, psum-row layout
CP_SEL = 1504                  # [128, 128] shift-up-64 + [64, 128] shift-dn-64
CPACK_W = 1760

F32 = mybir.dt.float32
BF16 = mybir.dt.bfloat16
GELU_A = 1.702
# CoreSim doesn't implement Gelu_apprx_sigmoid; substitute Sigmoid for
# structure-only sim runs (numerics then checked on HW via --randup).
ACT_QGELU = (
    mybir.ActivationFunctionType.Sigmoid
    if os.environ.get("KERNEL_DEBUG_SIM_ACT") == "1"
    else mybir.ActivationFunctionType.Gelu_apprx_sigmoid
)

_CACHE = {}


def build_kernel():
    if "nc" in _CACHE:
        return _CACHE["nc"]

    nc = bacc.Bacc("TRN2", target_bir_lowering=False, debug=False)

    x_d = nc.dram_tensor("x", [B_LOC, 128, KCH * PIX], BF16, kind="ExternalInput")
    cpk_d = nc.dram_tensor("cpack", [128, CPACK_W], BF16, kind="ExternalInput")
    cf_d = nc.dram_tensor("cf32", [128, 6], F32, kind="ExternalInput")
    # host-packed hypernet: [128, i, ol, t]; rows 0-63 = o<32, 64-127 = o>=32
    wh_d = nc.dram_tensor("w_hyper", [128, DIM * NHYP], BF16, kind="ExternalInput")
    out_d = nc.dram_tensor("out", [B_LOC, 128, KCH * PIX], BF16, kind="ExternalOutput")

    with tile.TileContext(nc) as tc:
        with tc.tile_pool(name="consts", bufs=1) as consts:
            # ---- constants: ONE cpack DMA (0.38MB, first on sync) + cf32;
            # everything else is a view into cpk_sb ----
            cpk_sb = consts.tile([128, CPACK_W], BF16)
            nc.sync.dma_start(cpk_sb[:], cpk_d[:])
            cf_sb = consts.tile([128, 6], F32)
            nc.sync.dma_start(cf_sb[:], cf_d[:])
            t2 = cpk_sb[:, CP_T2 : CP_T2 + 192]
            w_down2 = cpk_sb[:, CP_WDOWN : CP_WDOWN + 512].rearrange(
                "p (k m) -> p k m", k=KCH
            )
            b_prep2 = cpk_sb[:, CP_BPREP : CP_BPREP + NHYP]
            w_up65 = cpk_sb[:, CP_WUP : CP_WUP + 512]
            b_down2 = cf_sb[:, 0:1]
            scale_sb = cf_sb[0:DIM, 1:2]
            b_up_c = [cf_sb[:, 2 + kc : 3 + kc] for kc in range(KCH)]
            # 0/1 selector stationaries: matmuls route partitions, so the
            # hypernet-build partition moves never touch a DMA ring (ring
            # transfers were landing ~9us late behind the x-image backlog)
            sel_up = cpk_sb[:, CP_SEL : CP_SEL + 128]
            sel_dn = cpk_sb[0:DIM, CP_SEL + 128 : CP_SEL + 256]

            # PE warm-up: the HAM clock gate holds the PE at 1.2GHz until it
            # sees ~3.4us of sustained busy. Burn that window on dummy
            # matmuls over a scratch tile while the prologue DMAs stream, so
            # the hypernet and image matmuls all run at 2.4GHz. The two tiny
            # dummy activations preload the Gelu/Copy LUTs so the 1.3us
            # ACT_TABLE_LOADs don't land in the act critical path later.
            scratch = consts.tile([128, 128], BF16)
            nc.vector.memset(scratch[:], 0.25)
            dum = consts.tile([1, 16], BF16)
            nc.scalar.activation(
                dum[:], scratch[0:1, 0:16], ACT_QGELU, bias=0.0, scale=1.0
            )
            nc.scalar.copy(dum[:], scratch[0:1, 0:16])

            # w_conv2 top rows: W[i, (h, t, o32)] assembled from the
            # hypernet psum; only rows 0-63 are ever written/read (the
            # tap-pair stationaries live in wtap below)
            w_conv2 = consts.tile([128, DIM * 9], BF16)

            # ---- main pools ----
            with (
                tc.tile_pool(name="whpool", bufs=1) as whpool,
                tc.tile_pool(name="xin", bufs=4) as xin,
                tc.tile_pool(name="xact", bufs=4) as xactp,
                tc.tile_pool(name="xact2", bufs=4) as xact2p,
                tc.tile_pool(name="yact", bufs=3) as yactp,
                tc.tile_pool(name="tmp", bufs=6) as tmpp,
                tc.tile_pool(name="outs", bufs=2) as outsp,
                tc.tile_pool(name="ps_s", bufs=2, space="PSUM") as ps_sp,
                tc.tile_pool(name="ps_c", bufs=2, space="PSUM") as ps_cp,
                tc.tile_pool(name="ps_u", bufs=3, space="PSUM") as ps_up,
                tc.tile_pool(name="ps_h", bufs=1, space="PSUM") as ps_hp,
            ):
                # ---- prologue ----
                # warm-up burn (~3.6us of PE busy) in a dedicated psum bank
                # so the WAW chain of later heartbeats never blocks the
                # rotating up-proj psum tiles
                ps_w = ps_hp.tile([128, 128], F32, tag="hb", name="warm")
                for _ in range(34):
                    nc.tensor.matmul(
                        ps_w[:], scratch[:], scratch[:],
                        start=True, stop=True,
                    )

                def heartbeat(rhs64):
                    """Tiny matmul whose rhs is freshly-DMAed data: fires
                    exactly when that transfer lands, keeping the HAM
                    activity window non-idle through DMA-bound stretches
                    (an idle >3.4us re-throttles the PE to 1.2GHz, and it
                    has been observed stuck cold for 25us+ afterwards)."""
                    nc.tensor.matmul(
                        ps_w[0:64, 0:64], scratch[:, 0:64], rhs64,
                        start=True, stop=True,
                    )

                # one tile PER w_hyper chunk: tile-granular dependency
                # tracking would stall the first hypernet matmul until the
                # last chunk landed if this were a single tile
                def load_x(img, eng=None):
                    xT = xin.tile([128, KCH, PIX], BF16, tag="x", name=f"x{img}")
                    (eng or nc.sync).dma_start(
                        xT[:].rearrange("p k n -> p (k n)"), x_d[img]
                    )
                    heartbeat(xT[:, 0, 0:64])
                    return xT

                # queue split (each HWDGE data ring drains in FIFO
                # order, and triggers enter a ring the moment their deps
                # are ready -- so big loads must not be able to jump in
                # front of latency-critical small transfers):
                #   scalar ring: ALL wh chunks (drained ~27us), then only
                #     the small shift/build transfers (land promptly)
                #   sync ring:   cpack, cf32, x0.., stores (big streams)
                xTs = [load_x(0)]
                wh_chunks = []
                for q in range(NCHUNK):
                    cw = ICH * NHYP
                    t = whpool.tile([128, cw], BF16, tag=f"wh{q}")
                    eng = nc.scalar if q % 2 == 0 else nc.sync
                    eng.dma_start(t[:], wh_d[:, q * cw : (q + 1) * cw])
                    heartbeat(t[:, 0:64])
                    wh_chunks.append(t)
                xTs.append(load_x(1))
                xTs.append(load_x(2))

                def make_xacts(img):
                    """A/A2 padded buffers; one full memset of A-top zeroes
                    the pads (strided pad-only memsets measured slower on
                    gpsimd than one contiguous sweep); the act overwrites
                    the interior, the derived copies carry pads along."""
                    x_act = xactp.tile([128, PAD], BF16, tag="xa", name=f"xa{img}")
                    nc.gpsimd.memset(x_act[0:DIM, :], 0.0)
                    x_act2 = xact2p.tile([128, PAD], BF16, tag="xb", name=f"xb{img}")
                    return (x_act, x_act2)

                xacts = [make_xacts(0), make_xacts(1)]

                def wh_slice(il):
                    """rhs [128, 288] for hypernet row il, from its chunk."""
                    t = wh_chunks[il // ICH]
                    j = il % ICH
                    return t[:, j * NHYP : (j + 1) * NHYP]

                def down(img, xT):
                    """down-proj matmuls -> 2 psum tiles [128, 392]"""
                    ps_ds = [
                        ps_sp.tile([128, NHALF], F32, tag="pss", name=f"psd{img}_{rh}")
                        for rh in range(RH)
                    ]
                    for k in range(KCH):
                        for rh in range(RH):
                            nc.tensor.matmul(
                                ps_ds[rh][:],
                                w_down2[:, k, :],
                                xT[:, k, rh * NHALF : (rh + 1) * NHALF],
                                start=(k == 0),
                                stop=(k == KCH - 1),
                            )
                    return ps_ds

                def gelu1(img, ps_ds, x_act, x_act2):
                    """quickgelu(x+b) = Gelu_apprx_sigmoid(1.0*x + b) from
                    psum rows 0-63 into A-top (padded rows 1..28, cols
                    1..28)."""
                    x_act_v = x_act[:].rearrange("d (r c) -> d r c", c=PW)
                    for rh in range(RH):
                        ps_v = ps_ds[rh][:].rearrange("d (r c) -> d r c", c=W)
                        rows = slice(1 + rh * RROWS, 1 + (rh + 1) * RROWS)
                        nc.scalar.activation(
                            x_act_v[:DIM, rows, 1 : 1 + W],
                            ps_v[:DIM],
                            ACT_QGELU,
                            bias=b_down2[:DIM],
                            scale=1.0,
                        )
                    return (x_act_v, x_act2[:].rearrange("d (r c) -> d r c", c=PW))

                def shifts(img, x_act, x_act2):
                    """Derive the three other conv-packing copies from
                    A-top. conv(img) is ~1.5 image-blocks away, so these
                    can trail the gelu2/up work in each queue."""
                    # A-bottom: act shifted one flat element (one padded
                    # col); pad cols wrap into each other and carry zeros
                    nc.scalar.dma_start(
                        x_act[DIM:, 0 : PAD - 1], x_act[:DIM, 1:PAD]
                    )
                    # A2-top duplicates A-top verbatim (pads included);
                    # vector: gpsimd measured 3.2us for this copy vs 0.9
                    nc.vector.tensor_copy(x_act2[:DIM, :], x_act[:DIM, :])
                    # A2-bottom: act shifted one padded row-pair (2 rows =
                    # 60 flat elements)
                    nc.scalar.dma_start(
                        x_act2[DIM:, 0 : PAD - 2 * PW], x_act[:DIM, 2 * PW : PAD]
                    )

                # Prologue: downs go AFTER the first hypernet group in the
                # in-order PE queue (a stalled down matmul ahead of the
                # hypernet delays conv0 behind it).
                downed = []

                def issue_down(img, with_shifts=True):
                    ps = down(img, xTs[img])
                    xa = gelu1(img, ps, *xacts[img])
                    if with_shifts:
                        shifts(img, *xacts[img])
                    downed.append((ps, xa))

                # full hypernet: matmuls chase the streaming w_hyper DMA
                # chunk by chunk (region-level deps); psum/build work in two
                # 32-row groups (engine partition slices need 32 alignment).
                # Group g's psum rows [32g,32g+32) = W[i, o<32], rows
                # [64+32g, ..) = W[i, o>=32].
                t_b = tmpp.tile([128, NHYP], BF16, tag="t", name="t_b")
                for g in range(DIM // IGRP):
                    ps_q = ps_up.tile([128, NHYP], F32, tag="psu", name=f"hyp{g}")
                    for il in range(g * IGRP, (g + 1) * IGRP):
                        nc.tensor.matmul(
                            ps_q[:],
                            t2[:, 64 - il : 192 - il],
                            wh_slice(il),
                            start=(il % IGRP == 0),
                            stop=(il % IGRP == IGRP - 1),
                        )
                    if g == 0:
                        # two images' downs fill the w_hyper chunk-wait gaps
                        # between the hypernet groups
                        issue_down(0)
                        issue_down(1)
                    rt = slice(g * IGRP, (g + 1) * IGRP)
                    rb = slice(DIM + g * IGRP, DIM + (g + 1) * IGRP)
                    nc.vector.tensor_tensor(
                        w_conv2[rt, :NHYP], ps_q[rt, :], b_prep2[rt, :],
                        mybir.AluOpType.add,
                    )
                    nc.vector.tensor_tensor(
                        t_b[rb, :], ps_q[rb, :], b_prep2[rb, :],
                        mybir.AluOpType.add,
                    )
                # o>=32 block: PE routes t_b partitions 64+r -> r (matmul
                # operands must start at partition 0/32/64, so one K=64
                # routing matmul covers both groups), vector drains psum
                # into w_conv2 cols 288:576
                ps_m = ps_up.tile([128, NHYP], F32, tag="psu", name="mv")
                nc.tensor.matmul(
                    ps_m[:], sel_up[DIM:, :], t_b[DIM:, :],
                    start=True, stop=True,
                )
                nc.vector.tensor_copy(w_conv2[0:DIM, NHYP:], ps_m[0:DIM, :])
                # PE routes w_conv2 top rows r -> 64+r into psum (2 col
                # halves), then the 5 tap-pair stationaries are assembled
                # straight from SBUF-top/psum-bottom: wtap block rows 0-63 =
                # tap tA, rows 64-127 = tap tB (the pair contracted by one
                # K=128 matmul). 0..2 = (0,dx)+(2,dx), 3 = (1,0)+(1,1),
                # 4 = lone (1,2) (top half only). matmul lhsT APs may only
                # have ONE free dim, so these are contiguous [128, 64].
                ps_d = []
                for hh in range(2):
                    p = ps_up.tile([128, NHYP], F32, tag="psu", name=f"dn{hh}")
                    nc.tensor.matmul(
                        p[:], sel_dn[:, :],
                        w_conv2[0:DIM, hh * NHYP : (hh + 1) * NHYP],
                        start=True, stop=True,
                    )
                    ps_d.append(p)
                w2t = w_conv2[0:DIM, :].rearrange("p (h t o) -> p h t o", h=2, t=9)
                wtap = consts.tile([128, 5, 64], BF16)
                # lone-tap block: bottom 64 rows are ZERO so the matmul can
                # contract K=128 (K<=65 matmuls measured ~1.7x slower per
                # column than K=128)
                nc.vector.memset(wtap[DIM:, 4, :], 0.0)
                for j, (tA, tB) in enumerate(
                    [(0, 6), (1, 7), (2, 8), (3, 4), (5, None)]
                ):
                    nc.vector.tensor_copy(
                        wtap[0:DIM, j, :].rearrange("p (h o) -> p h o", h=2),
                        w2t[:, :, tA, :],
                    )
                    if tB is None:
                        continue
                    wb_v = wtap[DIM:, j, :].rearrange("p (h o) -> p h o", h=2)
                    for hh in range(2):
                        nc.vector.tensor_copy(
                            wb_v[:, hh, :],
                            ps_d[hh][DIM:, tB * 32 : (tB + 1) * 32],
                        )

                def conv(img):
                    """conv, 5 matmuls per half: 3 vertical pairs
                    (0,dx)+(2,dx) on A2/w_conv6, the pair (1,0)+(1,1) on
                    A/w_conv2, and the lone (1,2) tap at K=64"""
                    xact_cur, xact2_cur = downed[img][1]
                    ps_cs = []
                    for rh in range(RH):
                        ps_c = ps_cp.tile(
                            [DIM, NHALF], F32, tag="psc", name=f"psc{img}_{rh}"
                        )
                        for dx in range(3):
                            src = xact2_cur[
                                :, rh * RROWS : rh * RROWS + RROWS, dx : dx + W
                            ]
                            nc.tensor.matmul(
                                ps_c[:],
                                wtap[:, dx, :],
                                src,
                                start=(dx == 0),
                                stop=False,
                            )
                        nc.tensor.matmul(
                            ps_c[:],
                            wtap[:, 3, :],
                            xact_cur[
                                :, rh * RROWS + 1 : rh * RROWS + 1 + RROWS, 0:W
                            ],
                            start=False,
                            stop=False,
                        )
                        nc.tensor.matmul(
                            ps_c[:],
                            wtap[:, 4, :],
                            xact_cur[
                                :,
                                rh * RROWS + 1 : rh * RROWS + 1 + RROWS,
                                2 : 2 + W,
                            ],
                            start=False,
                            stop=True,
                        )
                        ps_cs.append(ps_c)
                    return ps_cs

                def up_part(img, o_sb, y_act, half, alternate=False):
                    """4 up matmuls (2 c-chunks) + psum->sbuf casts."""
                    for j, (kc, rh) in enumerate(
                        (kc, rh)
                        for kc in ((0, 1) if half == 0 else (2, 3))
                        for rh in range(RH)
                    ):
                        ps_u = ps_up.tile(
                            [128, NHALF], F32, tag="psu", name=f"psu{img}_{kc}_{rh}"
                        )
                        nc.tensor.matmul(
                            ps_u[:],
                            w_up65[:, kc * 128 : (kc + 1) * 128],
                            y_act[:, rh * NHALF : (rh + 1) * NHALF],
                            start=True,
                            stop=True,
                        )
                        dst = o_sb[:, kc, rh * NHALF : (rh + 1) * NHALF]
                        if (alternate and j % 2 == 1) or (
                            not alternate and half == 1 and kc == 3 and rh == 1
                        ):
                            nc.scalar.activation(
                                dst, ps_u[:],
                                mybir.ActivationFunctionType.Identity,
                                bias=b_up_c[kc], scale=1.0,
                            )
                        else:
                            nc.vector.tensor_scalar_add(dst, ps_u[:], b_up_c[kc])

                # Steady-state block for image i (software-pipelined):
                #   PE:     up_a(i-1), conv(i), up_b(i-1), down(i+2)
                #   scalar: gelu2(i), gelu1(i+2), A2-bottom trigger(i+2)
                #   vector: casts(i-1)
                #   gpsimd: y-ones(i), pad memsets(i+2), A2-top copy(i+2)
                #   sync:   load(i+3), store(i-1), A-bottom trigger(i+2)
                # Splitting up(i-1) around conv(i) gives the vector casts a
                # conv's worth of slack before the second psum rotation, so
                # the in-order PE queue never waits on a cast.
                y_acts = {}
                o_sbs = {}
                for img in range(B_LOC):
                    if img + 3 < B_LOC:
                        xTs.append(load_x(img + 3))
                    if img >= 1:
                        o_sbs[img - 1] = outsp.tile(
                            [128, KCH, PIX], BF16, tag="o", name=f"o{img-1}"
                        )
                        up_part(img - 1, o_sbs[img - 1], y_acts[img - 1], 0)

                    ps_cs = conv(img)

                    # gelu2: quickgelu(scale*y) = Gelu_apprx_sigmoid(scale*y)
                    # straight from psum into y_act (ones row fuses up bias)
                    y_act = yactp.tile([128, PIX], BF16, tag="ya")
                    y_acts[img] = y_act
                    nc.gpsimd.memset(y_act[DIM:, :], 0.0)
                    for rh in range(RH):
                        nc.scalar.activation(
                            y_act[:DIM, rh * NHALF : (rh + 1) * NHALF],
                            ps_cs[rh][:],
                            ACT_QGELU,
                            bias=0.0,
                            scale=scale_sb[:],
                        )

                    if img >= 1:
                        up_part(img - 1, o_sbs[img - 1], y_acts[img - 1], 1)
                        nc.sync.dma_start(
                            out_d[img - 1][:],
                            o_sbs[img - 1][:].rearrange("p k n -> p (k n)"),
                        )

                    if img + 2 < B_LOC:
                        xacts.append(make_xacts(img + 2))
                        issue_down(img + 2)

                # drain: last image's up with casts split across vector and
                # scalar (both idle now), store in two halves so the first
                # starts before the second is cast
                li = B_LOC - 1
                o_sb = outsp.tile([128, KCH, PIX], BF16, tag="o")
                up_part(li, o_sb, y_acts[li], 0, alternate=True)
                nc.sync.dma_start(
                    out_d[li][:, : 2 * PIX],
                    o_sb[:, 0:2, :].rearrange("p k n -> p (k n)"),
                )
                up_part(li, o_sb, y_acts[li], 1, alternate=True)
                nc.sync.dma_start(
                    out_d[li][:, 2 * PIX :],
                    o_sb[:, 2:4, :].rearrange("p k n -> p (k n)"),
                )

    nc.compile()
    _CACHE["nc"] = nc
    return nc


def _make_in_maps(inputs):
    bf16 = ml_dtypes.bfloat16
    x = np.ascontiguousarray(inputs["x"], dtype=np.float32)

    # ---- packed bf16 consts ----
    cpk = np.zeros((128, CPACK_W), dtype=bf16)
    wd = np.asarray(inputs["w_down"], np.float32).astype(bf16)
    t = wd.reshape(KCH, 128, DIM).transpose(1, 0, 2)       # [p, k, d]
    cpk[:, CP_WDOWN : CP_WDOWN + 512] = np.concatenate(
        [t, t], axis=2
    ).reshape(128, 512)
    cpk[0:DIM, CP_WUP : CP_WUP + 512] = np.asarray(
        inputs["w_up"], np.float32
    ).astype(bf16)
    cpk[DIM, CP_WUP : CP_WUP + 512] = np.asarray(
        inputs["b_up"], np.float32
    ).astype(bf16)
    emb = np.asarray(inputs["layer_emb"], np.float32).astype(bf16)
    cpk[0:EMB, CP_T2 + 64] = emb
    cpk[EMB:128, CP_T2 + 128] = emb
    bh = np.asarray(inputs["b_hyper"], np.float32).reshape(DIM, DIM, 9)
    b_ot = bh.transpose(1, 0, 2).astype(bf16)              # [i, o, t]
    cpk[0:DIM, CP_BPREP : CP_BPREP + NHYP] = (
        b_ot[:, :32].transpose(0, 2, 1).reshape(DIM, NHYP)   # [i, (t, ol)]
    )
    cpk[DIM:, CP_BPREP : CP_BPREP + NHYP] = (
        b_ot[:, 32:].transpose(0, 2, 1).reshape(DIM, NHYP)
    )

    sel = np.zeros((128, 256), dtype=bf16)
    for m in range(DIM):
        sel[m + DIM, m] = 1.0           # shift-up-64:  out row m <- row m+64
    for p in range(DIM):
        sel[p, 128 + DIM + p] = 1.0     # shift-dn-64:  out row p+64 <- row p
    cpk[:, CP_SEL:] = sel

    cf = np.zeros((128, 6), np.float32)
    bd = np.asarray(inputs["b_down"], np.float32)
    cf[0:DIM, 0] = bd
    cf[DIM:, 0] = bd
    cf[0:DIM, 1] = np.asarray(inputs["scale"], np.float32)
    bu = np.asarray(inputs["b_up"], np.float32)
    for kc in range(KCH):
        cf[:, 2 + kc] = bu[kc * 128 : (kc + 1) * 128]

    # ---- packed hypernet: [128, i, ol, t]; rows 0-63 = o<32 block ----
    wh = np.asarray(inputs["w_hyper"], np.float32).astype(bf16)
    wh = wh.reshape(EMB, DIM, DIM, 9)                      # [e, o, i, t]
    top = wh[:, :32].transpose(0, 2, 3, 1)                 # [e, i, t, ol]
    bot = wh[:, 32:].transpose(0, 2, 3, 1)
    whp = np.ascontiguousarray(
        np.concatenate([top, bot], axis=0).reshape(128, DIM * NHYP)
    )

    shared = {"cpack": cpk, "cf32": cf, "w_hyper": whp}
    in_maps = []
    for c in range(NCORES):
        xc = x[c * B_LOC : (c + 1) * B_LOC].reshape(B_LOC, PIX, KCH, 128)
        xt = np.ascontiguousarray(xc.transpose(0, 3, 2, 1)).astype(bf16)
        in_maps.append({"x": xt.reshape(B_LOC, 128, KCH * PIX), **shared})
    return in_maps


def _untranspose_out(res):
    outs = []
    for c in range(NCORES):
        o = np.asarray(res.results[c]["out"]).reshape(B_LOC, 128, KCH, PIX)
        o = o.transpose(0, 3, 2, 1).astype(np.float32)  # [img, pix, kc, p]
        outs.append(o.reshape(B_LOC, H, W, C))
    return np.concatenate(outs, axis=0)


def kernel(**inputs) -> np.ndarray:
    nc = build_kernel()
    in_maps = _make_in_maps(inputs)
    res = run_bass_kernel_spmd(nc, in_maps, core_ids=list(range(NCORES)))
    return _untranspose_out(res)


def run_traced(inputs, **kw):
    """For test.py: run with tracing to get HW exec time."""
    nc = build_kernel()
    in_maps = _make_in_maps(inputs)
    return run_bass_kernel_spmd(
        nc, in_maps, core_ids=list(range(NCORES)), trace=True, **kw
    )
